# revision 1
# baseline (speedup 1.0000x reference)
"""GAT (3-head, edge-weighted) message-passing kernel for 8 Trainium2 NeuronCores.

Strategy: sort edges by destination on host, give each core a contiguous
128-aligned destination-node range (49 windows x 128 nodes). Each core:
  phase 1: XT[n] = [x@W_lin+b | s_src(3) | s_dst(3)] (+ compact SDS[n,4] table)
  phase 2: per 128-edge tile: indirect-gather XT[src] and SDS[dst], softmax
           numerator p = exp(leakyrelu(s_src+s_dst)), one-hot scatter matmuls
           accumulate per-window denom [128,3] and agg [128f, 3*128n] in PSUM,
           per-window: project agg_h @ W_h scaled by 1/denom, write out rows.
No collectives needed: every core owns its dst range end-to-end.
"""

import numpy as np
import concourse.bass as bass
import concourse.bacc as bacc
import concourse.mybir as mybir
from concourse.tile import TileContext
from concourse import bass_utils

F32 = mybir.dt.float32
I32 = mybir.dt.int32

N_NODES = 50000
N_EDGES = 600000
DIM = 128
N_HEADS = 3
NEG_SLOPE = 0.2
NCORES = 8
NPW = 128                      # nodes per window
WPC = 49                       # windows per core
NPC = NPW * WPC                # 6272 nodes per core
NPAD = NPC * NCORES            # 50176
PADIDX = NPAD                  # poison row index (s_src = -1e4 -> p = 0)
NROWS = NPAD + 128             # 50304 rows in XT/SDS, 393 tiles of 128
NT1 = NROWS // 128             # phase-1 tiles

_cache = {}


def _phase1(nc, tc, cpool, xp, xt, sds, wg_sb, biasr_sb, poi_sb):
    with (
        tc.tile_pool(name="p1", bufs=6) as p1,
        tc.tile_pool(name="p1ps", bufs=4, space="PSUM") as p1ps,
    ):
        for i in range(NT1):
            xpt = p1.tile([128, DIM], F32, tag="xpt")
            nc.sync.dma_start(out=xpt[:], in_=xp[i * 128:(i + 1) * 128, :])
            ps = p1ps.tile([128, 138], F32, tag="ps1")
            nc.tensor.matmul(out=ps[:], lhsT=xpt[:], rhs=wg_sb[:],
                             start=True, stop=True)
            row = p1.tile([128, 138], F32, tag="row")
            nc.vector.tensor_tensor(out=row[:], in0=ps[:], in1=biasr_sb[:],
                                    op=mybir.AluOpType.add)
            nc.sync.dma_start(out=xt[i * 128:(i + 1) * 128, :],
                              in_=row[:, 0:134])
            nc.sync.dma_start(out=sds[i * 128:(i + 1) * 128, :],
                              in_=row[:, 134:138])
        # poison row: padding edges get s_src=-1e4 -> p = 0
        nc.sync.dma_start(out=xt[PADIDX:PADIDX + 1, 128:131], in_=poi_sb[:])


def _phase2(nc, tc, K, xt, sds, srca, wnid, dstc, ewa, ddia, outc,
            iota_sb, ident_sb, wh_sb, bias3_sb):
    with (
        tc.tile_pool(name="win", bufs=3) as wpool,
        tc.tile_pool(name="edge", bufs=16) as epool,
        tc.tile_pool(name="mst", bufs=6) as mpool,
        tc.tile_pool(name="oh", bufs=8) as ohpool,
        tc.tile_pool(name="sm", bufs=12) as smpool,
        tc.tile_pool(name="fl", bufs=3) as flpool,
        tc.tile_pool(name="aggps", bufs=2, space="PSUM") as aggps,
        tc.tile_pool(name="denps", bufs=2, space="PSUM") as denps,
        tc.tile_pool(name="sdps", bufs=2, space="PSUM") as sdps,
        tc.tile_pool(name="ops", bufs=2, space="PSUM") as ops,
    ):
        for w in range(WPC):
            r0 = w * 128
            srcw = wpool.tile([128, K], I32, tag="srcw")
            nc.sync.dma_start(out=srcw[:], in_=srca[r0:r0 + 128, :])
            dstcw = wpool.tile([128, K], F32, tag="dstcw")
            nc.sync.dma_start(out=dstcw[:], in_=dstc[r0:r0 + 128, :])
            eww = wpool.tile([128, K], F32, tag="eww")
            nc.sync.dma_start(out=eww[:], in_=ewa[r0:r0 + 128, :])
            ddiw = wpool.tile([128, K], F32, tag="ddiw")
            nc.sync.dma_start(out=ddiw[:], in_=ddia[r0:r0 + 128, :])
            wnw = wpool.tile([128, 1], I32, tag="wnw")
            nc.sync.dma_start(out=wnw[:], in_=wnid[r0:r0 + 128, :])
            sdw = wpool.tile([128, 4], F32, tag="sdw")
            nc.gpsimd.indirect_dma_start(
                out=sdw[:], out_offset=None, in_=sds[:],
                in_offset=bass.IndirectOffsetOnAxis(ap=wnw[:, 0:1], axis=0))
            ewn = wpool.tile([128, K], F32, tag="ewn")
            nc.vector.tensor_tensor(out=ewn[:], in0=eww[:], in1=ddiw[:],
                                    op=mybir.AluOpType.subtract)

            ps_den = denps.tile([128, 3], F32, tag="den")
            ps_agg = aggps.tile([128, N_HEADS * 128], F32, tag="agg")

            for t in range(K):
                xtg = epool.tile([128, 134], F32, tag="xtg")
                nc.gpsimd.indirect_dma_start(
                    out=xtg[:], out_offset=None, in_=xt[:],
                    in_offset=bass.IndirectOffsetOnAxis(
                        ap=srcw[:, t:t + 1], axis=0))
                onehot = ohpool.tile([128, 128], F32, tag="onehot")
                nc.vector.tensor_scalar(
                    out=onehot[:], in0=iota_sb[:],
                    scalar1=dstcw[:, t:t + 1], scalar2=None,
                    op0=mybir.AluOpType.is_equal)

                ps_mt = ops.tile([128, 128], F32, tag="pp")
                nc.tensor.transpose(out=ps_mt[:], in_=onehot[:],
                                    identity=ident_sb[:])
                mt = ohpool.tile([128, 128], F32, tag="mt")
                nc.scalar.activation(out=mt[:], in_=ps_mt[:],
                                     func=mybir.ActivationFunctionType.Copy)
                ps_sd = sdps.tile([128, 3], F32, tag="ps_sd")
                nc.tensor.matmul(out=ps_sd[:], lhsT=mt[:], rhs=sdw[:, 0:3],
                                 start=True, stop=True)
                e1 = smpool.tile([128, 3], F32, tag="e1")
                nc.vector.tensor_tensor(out=e1[:], in0=xtg[:, 128:131],
                                        in1=ps_sd[:],
                                        op=mybir.AluOpType.add)
                sc = smpool.tile([128, 3], F32, tag="sc")
                nc.vector.tensor_scalar(
                    out=sc[:], in0=e1[:], scalar1=NEG_SLOPE, scalar2=None,
                    op0=mybir.AluOpType.mult)
                t2 = smpool.tile([128, 3], F32, tag="t2")
                nc.vector.tensor_tensor(out=t2[:], in0=e1[:], in1=sc[:],
                                        op=mybir.AluOpType.max)
                p = smpool.tile([128, 3], F32, tag="p")
                nc.scalar.activation(out=p[:], in_=t2[:],
                                     func=mybir.ActivationFunctionType.Exp)

                nc.tensor.matmul(out=ps_den[:], lhsT=onehot[:], rhs=p[:],
                                 start=(t == 0), stop=(t == K - 1))

                q = smpool.tile([128, 3], F32, tag="q")
                nc.vector.tensor_scalar(
                    out=q[:], in0=p[:], scalar1=ewn[:, t:t + 1],
                    scalar2=None, op0=mybir.AluOpType.mult)

                mst = mpool.tile([128, N_HEADS * 128], F32, tag="mst")
                for h in range(2):
                    nc.vector.tensor_scalar(
                        out=mst[:, h * 128:(h + 1) * 128], in0=iota_sb[:],
                        scalar1=dstcw[:, t:t + 1],
                        scalar2=q[:, h:h + 1],
                        op0=mybir.AluOpType.is_equal,
                        op1=mybir.AluOpType.mult)
                # head 2 on ACT to offload DVE: onehot * q2
                nc.scalar.activation(out=mst[:, 256:384], in_=onehot[:],
                                     func=mybir.ActivationFunctionType.Copy,
                                     scale=q[:, 2:3])

                nc.tensor.matmul(out=ps_agg[:], lhsT=xtg[:, 0:128],
                                 rhs=mst[:], start=(t == 0),
                                 stop=(t == K - 1))

            # window flush
            den = flpool.tile([128, 3], F32, tag="dens")
            nc.vector.tensor_scalar(
                out=den[:], in0=ps_den[:], scalar1=1e-16, scalar2=3.0,
                op0=mybir.AluOpType.max, op1=mybir.AluOpType.mult)
            inv = flpool.tile([128, 3], F32, tag="inv")
            nc.vector.reciprocal(out=inv[:], in_=den[:])
            agg = flpool.tile([128, N_HEADS * 128], F32, tag="aggs")
            nc.scalar.activation(out=agg[:], in_=ps_agg[:],
                                 func=mybir.ActivationFunctionType.Copy)

            acc = flpool.tile([128, DIM], F32, tag="acc")
            tmp = flpool.tile([128, DIM], F32, tag="tmp")
            for h in range(N_HEADS):
                ps_o = ops.tile([128, DIM], F32, tag="pp")
                nc.tensor.matmul(out=ps_o[:],
                                 lhsT=agg[:, h * 128:(h + 1) * 128],
                                 rhs=wh_sb[:, h * DIM:(h + 1) * DIM],
                                 start=True, stop=True)
                dst_t = acc if h == 0 else tmp
                nc.vector.tensor_scalar(
                    out=dst_t[:], in0=ps_o[:], scalar1=inv[:, h:h + 1],
                    scalar2=None, op0=mybir.AluOpType.mult)
                if h > 0:
                    nc.vector.tensor_tensor(out=acc[:], in0=acc[:],
                                            in1=tmp[:],
                                            op=mybir.AluOpType.add)
            out_sb = flpool.tile([128, DIM], F32, tag="outsb")
            nc.vector.tensor_tensor(out=out_sb[:], in0=acc[:],
                                    in1=bias3_sb[:],
                                    op=mybir.AluOpType.add)
            nc.sync.dma_start(out=outc[r0:r0 + 128, :], in_=out_sb[:])


def _build(K):
    nc = bacc.Bacc("TRN2", target_bir_lowering=False, debug=False,
                   num_devices=NCORES)

    xp = nc.dram_tensor("xp", [NT1 * 128, DIM], F32, kind="ExternalInput")
    wg = nc.dram_tensor("wg", [DIM, 138], F32, kind="ExternalInput")
    biasr = nc.dram_tensor("biasr", [128, 138], F32, kind="ExternalInput")
    wheads = nc.dram_tensor("wheads", [N_HEADS * DIM, DIM], F32,
                            kind="ExternalInput")
    bias3 = nc.dram_tensor("bias3", [128, DIM], F32, kind="ExternalInput")
    iota = nc.dram_tensor("iota", [128, 128], F32, kind="ExternalInput")
    ident = nc.dram_tensor("ident", [128, 128], F32, kind="ExternalInput")
    poison = nc.dram_tensor("poison", [1, 3], F32, kind="ExternalInput")
    srca = nc.dram_tensor("srca", [WPC * 128, K], I32, kind="ExternalInput")
    wnid = nc.dram_tensor("wnid", [WPC * 128, 1], I32, kind="ExternalInput")
    dstg = nc.dram_tensor("dstg", [WPC * 128, K], I32, kind="ExternalInput")
    dstc = nc.dram_tensor("dstc", [WPC * 128, K], F32, kind="ExternalInput")
    ewa = nc.dram_tensor("ewa", [WPC * 128, K], F32, kind="ExternalInput")
    ddia = nc.dram_tensor("ddia", [WPC * 128, K], F32, kind="ExternalInput")

    xt = nc.dram_tensor("xt", [NROWS, 134], F32)
    sds = nc.dram_tensor("sds", [NROWS, 4], F32)
    outc = nc.dram_tensor("outc", [NPC, DIM], F32, kind="ExternalOutput")

    with TileContext(nc) as tc:
        with tc.tile_pool(name="const", bufs=1) as cpool:
            wg_sb = cpool.tile([DIM, 138], F32, tag="wg")
            nc.sync.dma_start(out=wg_sb[:], in_=wg[:])
            biasr_sb = cpool.tile([128, 138], F32, tag="biasr")
            nc.sync.dma_start(out=biasr_sb[:], in_=biasr[:])
            iota_sb = cpool.tile([128, 128], F32, tag="iota")
            nc.sync.dma_start(out=iota_sb[:], in_=iota[:])
            ident_sb = cpool.tile([128, 128], F32, tag="ident")
            nc.sync.dma_start(out=ident_sb[:], in_=ident[:])
            wh_sb = cpool.tile([128, N_HEADS * DIM], F32, tag="wh")
            for h in range(N_HEADS):
                nc.sync.dma_start(out=wh_sb[:, h * DIM:(h + 1) * DIM],
                                  in_=wheads[h * DIM:(h + 1) * DIM, :])
            bias3_sb = cpool.tile([128, DIM], F32, tag="bias3")
            nc.sync.dma_start(out=bias3_sb[:], in_=bias3[:])
            poi_sb = cpool.tile([1, 3], F32, tag="poi")
            nc.sync.dma_start(out=poi_sb[:], in_=poison[:])

            _phase1(nc, tc, cpool, xp, xt, sds, wg_sb, biasr_sb, poi_sb)
            _phase2(nc, tc, K, xt, sds, srca, wnid, dstc, ewa, ddia, outc,
                    iota_sb, ident_sb, wh_sb, bias3_sb)

    nc.compile()
    return nc


def _prep(x, edge_index, edge_ids, ddi_weight, W_lin, b_lin, edge_emb,
          W_heads, att_src, att_dst, bias_heads):
    x = np.asarray(x, np.float32)
    src = np.asarray(edge_index[0], np.int64)
    dst = np.asarray(edge_index[1], np.int64)
    eids = np.asarray(edge_ids, np.int64)
    ddi = np.asarray(ddi_weight, np.float32)
    W_lin = np.asarray(W_lin, np.float32)
    b_lin = np.asarray(b_lin, np.float32)
    edge_emb = np.asarray(edge_emb, np.float32)
    W_heads = np.asarray(W_heads, np.float32)
    att_src = np.asarray(att_src, np.float32)
    att_dst = np.asarray(att_dst, np.float32)
    bias_heads = np.asarray(bias_heads, np.float32)

    order = np.argsort(dst, kind="stable")
    src_s = src[order].astype(np.int32)
    dst_s = dst[order].astype(np.int32)
    ew0_s = edge_emb[eids[order], 0]
    ddi_s = ddi[order]

    bounds = np.searchsorted(dst_s, np.arange(0, NPAD + NPW, NPW))
    K = 1
    for c in range(NCORES):
        for w in range(WPC):
            wi = c * WPC + w
            K = max(K, (int(bounds[wi + 1] - bounds[wi]) + 127) // 128)

    per_core = []
    for c in range(NCORES):
        srca = np.full((WPC * 128, K), PADIDX, np.int32)
        dstga = np.zeros((WPC * 128, K), np.int32)
        dstca = np.zeros((WPC * 128, K), np.float32)
        ewa = np.zeros((WPC * 128, K), np.float32)
        ddia = np.zeros((WPC * 128, K), np.float32)
        for w in range(WPC):
            wi = c * WPC + w
            e0, e1 = int(bounds[wi]), int(bounds[wi + 1])
            n = e1 - e0
            base = wi * NPW
            dstga[w * 128:(w + 1) * 128, :] = base
            if n == 0:
                continue
            j = np.arange(n)
            pp = w * 128 + (j % 128)
            tt = j // 128
            srca[pp, tt] = src_s[e0:e1]
            dstga[pp, tt] = dst_s[e0:e1]
            dstca[pp, tt] = (dst_s[e0:e1] - base).astype(np.float32)
            ewa[pp, tt] = ew0_s[e0:e1]
            ddia[pp, tt] = ddi_s[e0:e1]
        wnid = (np.arange(WPC * 128, dtype=np.int32) + c * NPC)[:, None]
        per_core.append(dict(srca=srca, dstg=dstga, dstc=dstca,
                             ewa=ewa, ddia=ddia, wnid=wnid))

    # weight folding (host): scores s = x @ (W_lin@asd) + b@asd
    asd = np.zeros((DIM, 6), np.float32)
    for h in range(N_HEADS):
        asd[:, h] = W_heads[h] @ att_src[h]
        asd[:, 3 + h] = W_heads[h] @ att_dst[h]
    wg = np.zeros((DIM, 138), np.float32)
    wg[:, 0:128] = W_lin
    wg[:, 128:134] = W_lin @ asd
    wg[:, 134:137] = wg[:, 131:134]          # duplicate s_dst cols for SDS
    bias_ext = np.zeros(138, np.float32)
    bias_ext[0:128] = b_lin
    bias_ext[128:134] = b_lin @ asd
    bias_ext[134:137] = bias_ext[131:134]
    biasr = np.tile(bias_ext, (128, 1)).astype(np.float32)

    xpad = np.zeros((NT1 * 128, DIM), np.float32)
    xpad[:N_NODES] = x
    # phase-1 matmul lhsT must be x^T per 128-node tile
    xpt = np.zeros((NT1 * 128, DIM), np.float32)
    for i in range(NT1):
        xpt[i * 128:(i + 1) * 128] = xpad[i * 128:(i + 1) * 128].T
    wheads2 = W_heads.reshape(N_HEADS * DIM, DIM).copy()
    bias3 = np.tile(bias_heads.sum(0) / N_HEADS, (128, 1)).astype(np.float32)
    iota = np.tile(np.arange(128, dtype=np.float32), (128, 1))
    poisonv = np.full((1, 3), -1e4, np.float32)

    shared = dict(xp=xpt, wg=wg, biasr=biasr, wheads=wheads2, bias3=bias3,
                  iota=iota, ident=np.eye(128, dtype=np.float32),
                  poison=poisonv)
    in_maps = []
    for c in range(NCORES):
        m = dict(shared)
        m.update(per_core[c])
        in_maps.append(m)
    return K, in_maps


def kernel(**inputs):
    K, in_maps = _prep(**inputs)
    if K not in _cache:
        _cache[K] = _build(K)
    nc = _cache[K]
    res = bass_utils.run_bass_kernel_spmd(nc, in_maps,
                                          core_ids=list(range(NCORES)))
    out = np.concatenate([res.results[c]["outc"] for c in range(NCORES)],
                         axis=0)
    return np.ascontiguousarray(out[:N_NODES]).astype(np.float32)



# revision 13
# speedup vs baseline: 3.9677x; 3.9677x over previous
"""GAT (3-head, edge-weighted) message-passing kernel for 8 Trainium2 NeuronCores.

v2 design (bf16 + dma_gather + folded weights):
  Host folds W_lin into everything: scores s = x @ (W_lin W_h a), messages
  aggregate RAW x and project by W' = W_lin W_h / 3 at the end. Node rows
  live in XROW [50178, 256] bf16 = [x(128) | s_src(3) | s_dst(3) | pad],
  512B rows (dma_gather elem multiple of 256B). Rows are split into two
  banks of <=25089 rows with a poison row 0 each so gather indices fit in
  int16; pad slots index the all-zero poison row.

  Phase 1 (per core, replicated): 8 slab loads of xT bf16, 392 tiny matmuls
  x_tile @ asd6 -> 6 score cols, written into XROW score columns.

  Phase 2: edges sorted by dst; each core owns 49 windows of 128 dst nodes.
  Windows are permuted per-core (sorted by slot count) against a shared
  rank-max schedule so the SPMD instruction stream is identical. Per window:
  one dma_gather per bank pulls all K*128 edge rows; onehot machinery is
  built with 4x-mode bf16 tensor_scalar ops; per 128-edge slot: 3 matmuls
  (agg [128,384], den [128,3], s_dst-broadcast [128,3]) accumulate in PSUM.
  Flush: reciprocal of den, per-head projection by wfold, ACT-scaled sum.
"""

import numpy as np
import ml_dtypes

import concourse.bass as bass
import concourse.bacc as bacc
import concourse.mybir as mybir
from concourse.tile import TileContext
from concourse import bass_utils

F32 = mybir.dt.float32
BF16 = mybir.dt.bfloat16
I32 = mybir.dt.int32
I16 = mybir.dt.int16
BF_NP = ml_dtypes.bfloat16

N_NODES = 50000
N_EDGES = 600000
DIM = 128
NH = 3
NEG = 0.2

NCORES = 8
NPW = 128                    # dst nodes per window
WPC = 49                     # windows per core
NPC = NPW * WPC              # 6272
NTILES = 392                 # node tiles of 128 (50176 nodes incl pad)
NROWS = NTILES * 128         # 50176
ABANK = NROWS // 2           # nodes 0..25087 -> bank A
AROWS = ABANK + 1            # poison row + 25088 node rows
XR = 2 * AROWS               # 50178 total XROW rows
ROWC = 256                   # bf16 cols per row (512B)
SLABS = 8
TPS = NTILES // SLABS        # 49 tiles per slab
KMAX_TILE = 32               # dstcT tile partition allocation

_cache = {}


def _row_of(n):
    """XROW row of node n (vectorized)."""
    n = np.asarray(n)
    return np.where(n < ABANK, 1 + n, AROWS + 1 + (n - ABANK)).astype(np.int64)


def _build(sched, has_obias, has_bmsg, kmax):
    KAs, KBs = sched
    Ks = [a + b for a, b in zip(KAs, KBs)]
    off8 = np.concatenate([[0], np.cumsum([8 * k for k in Ks])]).astype(int)
    off2 = np.concatenate([[0], np.cumsum([2 * k for k in Ks])]).astype(int)
    off128 = np.concatenate([[0], np.cumsum([128 * k for k in Ks])]).astype(int)

    nc = bacc.Bacc("TRN2", target_bir_lowering=False, debug=False,
                   num_devices=NCORES)

    xrow = nc.dram_tensor("xrow", [XR, ROWC], BF16, kind="ExternalInput")
    xt = nc.dram_tensor("xt", [128, NROWS], BF16, kind="ExternalInput")
    asd6 = nc.dram_tensor("asd6", [128, 6], BF16, kind="ExternalInput")
    biasr42 = nc.dram_tensor("biasr42", [128, 42], BF16, kind="ExternalInput")
    wfold = nc.dram_tensor("wfold", [128, NH * DIM], BF16,
                           kind="ExternalInput")
    iota = nc.dram_tensor("iota", [128, 128], BF16, kind="ExternalInput")
    obias = nc.dram_tensor("obias", [128, DIM], F32, kind="ExternalInput")
    bmsg3 = nc.dram_tensor("bmsg3", [NH, DIM], F32, kind="ExternalInput")
    idx_all = nc.dram_tensor("idx_all", [128, off8[-1]], I16,
                             kind="ExternalInput")
    meta_all = nc.dram_tensor("meta_all", [128, off2[-1]], F32,
                              kind="ExternalInput")
    dstcF_all = nc.dram_tensor("dstcF_all", [1, off128[-1]], BF16,
                               kind="ExternalInput")
    iotacol = nc.dram_tensor("iotacol", [128, 1], F32, kind="ExternalInput")
    wnid = nc.dram_tensor("wnid", [NPC, 1], I32, kind="ExternalInput")
    outc = nc.dram_tensor("outc", [NPC, DIM], F32, kind="ExternalOutput")

    with TileContext(nc) as tc:
        with tc.tile_pool(name="const", bufs=1) as cpool:
            asd_sb = cpool.tile([128, 6], BF16, tag="asd")
            nc.sync.dma_start(out=asd_sb[:], in_=asd6[:])
            biasr_sb = cpool.tile([128, 42], BF16, tag="biasr")
            nc.sync.dma_start(out=biasr_sb[:], in_=biasr42[:])
            wf_sb = cpool.tile([128, NH * DIM], BF16, tag="wf")
            nc.sync.dma_start(out=wf_sb[:], in_=wfold[:])
            iota_sb = cpool.tile([128, 128], BF16, tag="iota")
            nc.sync.dma_start(out=iota_sb[:], in_=iota[:])
            iotacol_sb = cpool.tile([128, 1], F32, tag="iotacol")
            nc.sync.dma_start(out=iotacol_sb[:], in_=iotacol[:])
            obias_sb = cpool.tile([128, DIM], F32, tag="obias")
            if has_obias:
                nc.sync.dma_start(out=obias_sb[:], in_=obias[:])
            bmsg_sb = cpool.tile([NH, DIM], F32, tag="bmsg")
            if has_bmsg:
                nc.sync.dma_start(out=bmsg_sb[:], in_=bmsg3[:])

            # ---------------- phase 1: scores ----------------
            with (
                tc.tile_pool(name="p1", bufs=2) as p1,
                tc.tile_pool(name="p1s", bufs=3) as p1s,
                tc.tile_pool(name="p1ps", bufs=2, space="PSUM") as p1ps,
            ):
                for s in range(SLABS):
                    slab = p1.tile([128, TPS * 128], BF16, tag="slab")
                    nc.sync.dma_start(
                        out=slab[:], in_=xt[:, s * TPS * 128:(s + 1) * TPS * 128])
                    scr = p1s.tile([128, TPS * 6], BF16, tag="scr")
                    for g in range(7):          # 7 groups of 7 tiles
                        ps = p1ps.tile([128, 42], F32, tag="ps")
                        for t in range(7):
                            ti = g * 7 + t
                            nc.tensor.matmul(
                                out=ps[:, 6 * t:6 * t + 6],
                                lhsT=slab[:, ti * 128:(ti + 1) * 128],
                                rhs=asd_sb[:], start=True, stop=True)
                        nc.vector.tensor_tensor(
                            out=scr[:, g * 42:(g + 1) * 42], in0=ps[:],
                            in1=biasr_sb[:], op=mybir.AluOpType.add)
                    row0 = (1 + s * TPS * 128 if s < SLABS // 2
                            else AROWS + 1 + (s - SLABS // 2) * TPS * 128)
                    dst_ap = xrow[row0:row0 + TPS * 128, 128:134].rearrange(
                        "(t p) c -> p t c", p=128)
                    src_ap = scr[:].rearrange("p (t c) -> p t c", c=6)
                    nc.sync.dma_start(out=dst_ap, in_=src_ap)

            # ---------------- phase 2: windows ----------------
            with (
                tc.tile_pool(name="gt", bufs=2) as gpool,
                tc.tile_pool(name="win", bufs=3) as wpool,
                tc.tile_pool(name="ot", bufs=3) as otpool,
                tc.tile_pool(name="sm", bufs=3) as smpool,
                tc.tile_pool(name="mst", bufs=4) as mstpool,
                tc.tile_pool(name="fl", bufs=2) as flpool,
                tc.tile_pool(name="stg", bufs=2) as stgpool,
                tc.tile_pool(name="aggps", bufs=2, space="PSUM") as aggps_p,
                tc.tile_pool(name="denps", bufs=2, space="PSUM") as denps_p,
                tc.tile_pool(name="sdps", bufs=2, space="PSUM") as sdps_p,
                tc.tile_pool(name="ops", bufs=2, space="PSUM") as ops_p,
            ):
                stg = None
                for j in range(WPC):
                    KA, KB = KAs[j], KBs[j]
                    K = KA + KB
                    meta = wpool.tile([128, 2 * kmax], F32, tag="meta")
                    nc.sync.dma_start(
                        out=meta[:, 0:2 * K],
                        in_=meta_all[:, off2[j]:off2[j] + 2 * K])
                    idxt = wpool.tile([128, 8 * kmax], I16, tag="idx")
                    nc.sync.dma_start(
                        out=idxt[:, 0:8 * K],
                        in_=idx_all[:, off8[j]:off8[j] + 8 * K])
                    dstcF = wpool.tile([1, kmax * 128], BF16, tag="dstcF")
                    nc.sync.dma_start(
                        out=dstcF[0:1, 0:K * 128],
                        in_=dstcF_all[:, off128[j]:off128[j] + K * 128])
                    dstcB = wpool.tile([128, kmax * 128], BF16, tag="dstcB")
                    nc.gpsimd.partition_broadcast(
                        out_ap=dstcB[:, 0:K * 128], in_ap=dstcF[0:1, 0:K * 128])
                    wn = wpool.tile([128, 1], I32, tag="wn")
                    nc.sync.dma_start(out=wn[:], in_=wnid[j * 128:(j + 1) * 128, :])
                    sdw = wpool.tile([128, ROWC], BF16, tag="sdw")
                    nc.gpsimd.indirect_dma_start(
                        out=sdw[:], out_offset=None, in_=xrow[:],
                        in_offset=bass.IndirectOffsetOnAxis(ap=wn[:, 0:1], axis=0))

                    gt = gpool.tile([128, kmax * ROWC], BF16, tag="gt")
                    gv = gt[:, 0:K * ROWC].rearrange("p (k c) -> p k c", c=ROWC)
                    if KA > 0:
                        nc.gpsimd.dma_gather(
                            gv[:, 0:KA, :], xrow[0:AROWS, :],
                            idxt[:, 0:8 * KA], KA * 128, KA * 128, ROWC)
                    if KB > 0:
                        nc.gpsimd.dma_gather(
                            gv[:, KA:K, :], xrow[AROWS:XR, :],
                            idxt[:, 8 * KA:8 * K], KB * 128, KB * 128, ROWC)

                    # s_dst per edge: oT[d,e] = (dstcT[k,e]==d); sd = oT^T @ sdw
                    sdps = sdps_p.tile([128, 3 * kmax], F32, tag="sdps")
                    for k in range(K):
                        oT = otpool.tile([128, 128], BF16, tag="oT")
                        nc.vector.tensor_scalar(
                            out=oT[:], in0=dstcB[:, k * 128:(k + 1) * 128],
                            scalar1=iotacol_sb[:, 0:1], scalar2=None,
                            op0=mybir.AluOpType.is_equal)
                        nc.tensor.matmul(
                            out=sdps[:, 3 * k:3 * k + 3], lhsT=oT[:],
                            rhs=sdw[:, 131:134], start=True, stop=True)

                    # scores: e = lrelu(s_src + s_dst); p = exp(e); q = p*ew
                    e1 = smpool.tile([128, 3 * kmax], BF16, tag="e1")
                    nc.vector.tensor_tensor(
                        out=e1[:, 0:3 * K].rearrange("p (k c) -> p k c", c=3),
                        in0=gv[:, :, 128:131],
                        in1=sdps[:, 0:3 * K].rearrange("p (k c) -> p k c", c=3),
                        op=mybir.AluOpType.add)
                    sl = smpool.tile([128, 3 * kmax], BF16, tag="sl")
                    nc.vector.tensor_scalar(
                        out=sl[:, 0:3 * K], in0=e1[:, 0:3 * K], scalar1=NEG,
                        scalar2=None, op0=mybir.AluOpType.mult)
                    e2 = smpool.tile([128, 3 * kmax], BF16, tag="e2")
                    nc.vector.tensor_tensor(
                        out=e2[:, 0:3 * K], in0=e1[:, 0:3 * K],
                        in1=sl[:, 0:3 * K], op=mybir.AluOpType.max)
                    p_all = smpool.tile([128, 3 * kmax], BF16, tag="p")
                    nc.scalar.activation(
                        out=p_all[:, 0:3 * K], in_=e2[:, 0:3 * K],
                        func=mybir.ActivationFunctionType.Exp)
                    q_all = smpool.tile([128, 3 * kmax], F32, tag="q")
                    nc.vector.tensor_tensor(
                        out=q_all[:, 0:3 * K].rearrange("p (k c) -> p k c", c=3),
                        in0=p_all[:, 0:3 * K].rearrange("p (k c) -> p k c", c=3),
                        in1=meta[:, K:2 * K].unsqueeze(2).broadcast_to(
                            (128, K, 3)),
                        op=mybir.AluOpType.mult)

                    aggps = aggps_p.tile([128, NH * 128], F32, tag="agg")
                    denps = denps_p.tile([128, 8], F32, tag="den")
                    for k in range(K):
                        mst = mstpool.tile([128, 512], BF16, tag="mst")
                        dcol = meta[:, k:k + 1]
                        nc.vector.tensor_scalar(
                            out=mst[:, 384:512], in0=iota_sb[:],
                            scalar1=dcol, scalar2=None,
                            op0=mybir.AluOpType.is_equal)
                        for h in range(2):
                            nc.vector.tensor_scalar(
                                out=mst[:, h * 128:(h + 1) * 128],
                                in0=iota_sb[:], scalar1=dcol,
                                scalar2=q_all[:, 3 * k + h:3 * k + h + 1],
                                op0=mybir.AluOpType.is_equal,
                                op1=mybir.AluOpType.mult)
                        nc.scalar.activation(
                            out=mst[:, 256:384], in_=mst[:, 384:512],
                            func=mybir.ActivationFunctionType.Copy,
                            scale=q_all[:, 3 * k + 2:3 * k + 3])
                        nc.tensor.matmul(
                            out=aggps[:], lhsT=gt[:, k * ROWC:k * ROWC + 128],
                            rhs=mst[:, 0:384], start=(k == 0), stop=(k == K - 1))
                        nc.tensor.matmul(
                            out=denps[:, 0:3], lhsT=mst[:, 384:512],
                            rhs=p_all[:, 3 * k:3 * k + 3],
                            start=(k == 0), stop=(k == K - 1))
                        if has_bmsg:
                            nc.tensor.matmul(
                                out=denps[:, 3:6], lhsT=mst[:, 384:512],
                                rhs=q_all[:, 3 * k:3 * k + 3],
                                start=(k == 0), stop=(k == K - 1))

                    # ---- window flush ----
                    dinv = flpool.tile([128, 3], F32, tag="dinv")
                    nc.vector.tensor_scalar(
                        out=dinv[:], in0=denps[:, 0:3], scalar1=1e-16,
                        scalar2=None, op0=mybir.AluOpType.max)
                    nc.vector.reciprocal(out=dinv[:], in_=dinv[:])
                    aggsb = flpool.tile([128, NH * 128], BF16, tag="aggsb")
                    nc.scalar.activation(
                        out=aggsb[:], in_=aggps[:],
                        func=mybir.ActivationFunctionType.Copy)
                    ths = []
                    for h in range(NH):
                        o2 = ops_p.tile([128, DIM], F32, tag="o2")
                        nc.tensor.matmul(
                            out=o2[:], lhsT=aggsb[:, h * 128:(h + 1) * 128],
                            rhs=wf_sb[:, h * DIM:(h + 1) * DIM],
                            start=True, stop=True)
                        th = flpool.tile([128, DIM], F32, tag=f"th{h}")
                        nc.scalar.activation(
                            out=th[:], in_=o2[:],
                            func=mybir.ActivationFunctionType.Copy,
                            scale=dinv[:, h:h + 1])
                        ths.append(th)
                    if j % 7 == 0:
                        stg = stgpool.tile([128, 7 * DIM], F32, tag="stg")
                    sg = stg[:, (j % 7) * DIM:(j % 7 + 1) * DIM]
                    a01 = flpool.tile([128, DIM], F32, tag="a01")
                    nc.vector.tensor_tensor(out=a01[:], in0=ths[0][:],
                                            in1=ths[1][:],
                                            op=mybir.AluOpType.add)
                    if has_bmsg:
                        a01b = flpool.tile([128, DIM], F32, tag="a01b")
                        nc.vector.tensor_tensor(out=a01b[:], in0=a01[:],
                                                in1=ths[2][:],
                                                op=mybir.AluOpType.add)
                        tn = flpool.tile([128, 3], F32, tag="tn")
                        nc.vector.tensor_tensor(out=tn[:], in0=denps[:, 3:6],
                                                in1=dinv[:],
                                                op=mybir.AluOpType.mult)
                        bc = flpool.tile([128, DIM], F32, tag="bc")
                        for h in range(NH):
                            dst_t = bc if h == 0 else flpool.tile(
                                [128, DIM], F32, tag="bch")
                            nc.vector.tensor_scalar(
                                out=dst_t[:],
                                in0=bmsg_sb[h:h + 1, :].partition_broadcast(128),
                                scalar1=tn[:, h:h + 1], scalar2=None,
                                op0=mybir.AluOpType.mult)
                            if h > 0:
                                nc.vector.tensor_tensor(
                                    out=bc[:], in0=bc[:], in1=dst_t[:],
                                    op=mybir.AluOpType.add)
                        if has_obias:
                            nc.vector.tensor_tensor(
                                out=bc[:], in0=bc[:], in1=obias_sb[:],
                                op=mybir.AluOpType.add)
                        nc.vector.tensor_tensor(out=sg, in0=a01b[:],
                                                in1=bc[:],
                                                op=mybir.AluOpType.add)
                    else:
                        if has_obias:
                            a2 = flpool.tile([128, DIM], F32, tag="a2")
                            nc.vector.tensor_tensor(out=a2[:], in0=ths[2][:],
                                                    in1=obias_sb[:],
                                                    op=mybir.AluOpType.add)
                            nc.vector.tensor_tensor(out=sg, in0=a01[:],
                                                    in1=a2[:],
                                                    op=mybir.AluOpType.add)
                        else:
                            nc.vector.tensor_tensor(out=sg, in0=a01[:],
                                                    in1=ths[2][:],
                                                    op=mybir.AluOpType.add)
                    if j % 7 == 6:
                        j0 = j - 6
                        dst_ap = outc[j0 * 128:(j0 + 7) * 128, :].rearrange(
                            "(g p) c -> p g c", p=128)
                        nc.sync.dma_start(
                            out=dst_ap,
                            in_=stg[:].rearrange("p (g c) -> p g c", c=DIM))

    nc.compile()
    return nc


def _prep(x, edge_index, edge_ids, ddi_weight, W_lin, b_lin, edge_emb,
          W_heads, att_src, att_dst, bias_heads):
    x = np.asarray(x, np.float32)
    src = np.asarray(edge_index[0], np.int64)
    dst = np.asarray(edge_index[1], np.int64)
    eids = np.asarray(edge_ids, np.int64)
    ddi = np.asarray(ddi_weight, np.float32)
    W_lin = np.asarray(W_lin, np.float32)
    b_lin = np.asarray(b_lin, np.float32)
    edge_emb = np.asarray(edge_emb, np.float32)
    W_heads = np.asarray(W_heads, np.float32)
    att_src = np.asarray(att_src, np.float32)
    att_dst = np.asarray(att_dst, np.float32)
    bias_heads = np.asarray(bias_heads, np.float32)

    # ---- folded weights ----
    asd = np.zeros((DIM, 6), np.float32)
    sbias = np.zeros(6, np.float32)
    for h in range(NH):
        wsrc = W_lin @ W_heads[h] @ att_src[h]
        wdst = W_lin @ W_heads[h] @ att_dst[h]
        asd[:, h] = wsrc
        asd[:, 3 + h] = wdst
        sbias[h] = b_lin @ W_heads[h] @ att_src[h]
        sbias[3 + h] = b_lin @ W_heads[h] @ att_dst[h]
    wfold = np.zeros((DIM, NH * DIM), np.float32)
    bmsg3 = np.zeros((NH, DIM), np.float32)
    for h in range(NH):
        wfold[:, h * DIM:(h + 1) * DIM] = (W_lin @ W_heads[h]) / NH
        bmsg3[h] = (b_lin @ W_heads[h]) / NH
    obias = np.tile(bias_heads.sum(0) / NH, (128, 1)).astype(np.float32)
    has_obias = bool(np.abs(obias).max() > 0)
    has_bmsg = bool(np.abs(bmsg3).max() > 0)

    # ---- node rows ----
    xb = np.zeros((NROWS, DIM), BF_NP)
    xb[:N_NODES] = x.astype(BF_NP)
    xrow = np.zeros((XR, ROWC), BF_NP)
    xrow[1:AROWS, 0:DIM] = xb[0:ABANK]
    xrow[AROWS + 1:XR, 0:DIM] = xb[ABANK:]
    xt = np.ascontiguousarray(xb.T).astype(BF_NP)        # [128, NROWS]

    # ---- edges: sort by dst, split per window by src bank ----
    order = np.argsort(dst, kind="stable")
    src_s = src[order].astype(np.int64)
    dst_s = dst[order].astype(np.int64)
    ew_s = (edge_emb[eids[order], 0] - ddi[order]).astype(np.float32)
    bounds = np.searchsorted(dst_s, np.arange(0, NROWS + NPW, NPW))

    WN = NCORES * WPC
    winA, winB = [], []
    for wi in range(WN):
        e0, e1 = int(bounds[wi]), int(bounds[wi + 1])
        s_, d_, w_ = src_s[e0:e1], dst_s[e0:e1], ew_s[e0:e1]
        am = s_ < ABANK
        winA.append((s_[am], d_[am], w_[am]))
        winB.append((s_[~am], d_[~am], w_[~am]))

    def kcnt(n):
        return (n + 127) // 128

    # per-core window permutation: sort by total slot count desc
    perms = []
    for c in range(NCORES):
        ks = [kcnt(len(winA[c * WPC + j][0])) + kcnt(len(winB[c * WPC + j][0]))
              for j in range(WPC)]
        perms.append(np.argsort(-np.asarray(ks), kind="stable"))
    KAs, KBs = [], []
    for j in range(WPC):
        KAs.append(max(kcnt(len(winA[c * WPC + perms[c][j]][0]))
                       for c in range(NCORES)))
        KBs.append(max(kcnt(len(winB[c * WPC + perms[c][j]][0]))
                       for c in range(NCORES)))
    kmax = max(a + b for a, b in zip(KAs, KBs))
    sched = (tuple(KAs), tuple(KBs))
    Ks = [a + b for a, b in zip(KAs, KBs)]
    off8 = np.concatenate([[0], np.cumsum([8 * k for k in Ks])]).astype(int)
    off2 = np.concatenate([[0], np.cumsum([2 * k for k in Ks])]).astype(int)
    KTOT = int(sum(Ks))
    assert kmax <= KMAX_TILE

    iota = np.tile(np.arange(128, dtype=np.float32), (128, 1)).astype(BF_NP)
    iotacol = np.arange(128, dtype=np.float32)[:, None]
    off128 = np.concatenate([[0], np.cumsum([128 * k for k in Ks])]).astype(int)
    biasr42 = np.tile(sbias, (128, 7)).astype(BF_NP)

    shared = dict(
        xrow=xrow, xt=xt, asd6=asd.astype(BF_NP), biasr42=biasr42,
        wfold=wfold.astype(BF_NP), iota=iota, iotacol=iotacol, obias=obias,
        bmsg3=bmsg3)

    in_maps = []
    perm_list = []
    for c in range(NCORES):
        idx_all = np.zeros((128, off8[-1]), np.int16)
        meta_all = np.zeros((128, off2[-1]), np.float32)
        dstcF_all = np.full((1, off128[-1]), 128.0, BF_NP)
        wnid_a = np.zeros((NPC, 1), np.int32)
        for j in range(WPC):
            gw = c * WPC + int(perms[c][j])
            KA, KB, K = KAs[j], KBs[j], Ks[j]
            base = gw * NPW
            dstc = np.full((128, K), 128.0, np.float32)
            ewn = np.zeros((128, K), np.float32)
            idxs = np.zeros(8 * K * 16, np.int64).reshape(16, 8 * K)

            for (s_, d_, w_), k0, rowoff in (
                    (winA[gw], 0, 1),
                    (winB[gw], KA, 1 - ABANK)):
                n = len(s_)
                if n == 0:
                    continue
                i = np.arange(n)
                p = i % 128
                k = k0 + i // 128
                dstc[p, k] = (d_ - base).astype(np.float32)
                ewn[p, k] = w_
                rows = (s_ + rowoff).astype(np.int64)   # bank-local row id
                # gather-local position i -> idx slot [i%16, 8*k0*2 + i//16]
                idxs[i % 16, 8 * k0 + i // 16] = rows
            # Q7 gather reads the idx stream from one 16-partition group
            # (which one depends on the queue) — replicate to all 8 groups.
            idx_all[:, off8[j]:off8[j + 1]] = np.tile(
                idxs.astype(np.int16), (8, 1))
            meta_all[:, off2[j]:off2[j] + K] = dstc
            meta_all[:, off2[j] + K:off2[j] + 2 * K] = ewn
            dstcF_all[0, off128[j]:off128[j] + K * 128] = \
                dstc.T.reshape(-1).astype(BF_NP)
            wnid_a[j * 128:(j + 1) * 128, 0] = _row_of(
                np.arange(base, base + NPW))
        m = dict(shared)
        m.update(idx_all=idx_all, meta_all=meta_all, dstcF_all=dstcF_all,
                 wnid=wnid_a)
        in_maps.append(m)
        perm_list.append(perms[c])

    key = (sched, has_obias, has_bmsg)
    return key, in_maps, perm_list


def kernel(**inputs):
    key, in_maps, perm_list = _prep(**inputs)
    sched, has_obias, has_bmsg = key
    kmax = max(a + b for a, b in zip(sched[0], sched[1]))
    if key not in _cache:
        _cache[key] = _build(sched, has_obias, has_bmsg, kmax)
    nc = _cache[key]
    res = bass_utils.run_bass_kernel_spmd(nc, in_maps,
                                          core_ids=list(range(NCORES)))
    out = np.zeros((NROWS, DIM), np.float32)
    for c in range(NCORES):
        oc = res.results[c]["outc"]
        for j in range(WPC):
            gw = c * WPC + int(perm_list[c][j])
            out[gw * NPW:(gw + 1) * NPW] = oc[j * 128:(j + 1) * 128]
    return np.ascontiguousarray(out[:N_NODES]).astype(np.float32)


# revision 14
# speedup vs baseline: 4.1980x; 1.0580x over previous
"""GAT (3-head, edge-weighted) message-passing kernel for 8 Trainium2 NeuronCores.

v2 design (bf16 + dma_gather + folded weights):
  Host folds W_lin into everything: scores s = x @ (W_lin W_h a), messages
  aggregate RAW x and project by W' = W_lin W_h / 3 at the end. Node rows
  live in XROW [50178, 256] bf16 = [x(128) | s_src(3) | s_dst(3) | pad],
  512B rows (dma_gather elem multiple of 256B). Rows are split into two
  banks of <=25089 rows with a poison row 0 each so gather indices fit in
  int16; pad slots index the all-zero poison row.

  Phase 1 (per core, replicated): 8 slab loads of xT bf16, 392 tiny matmuls
  x_tile @ asd6 -> 6 score cols, written into XROW score columns.

  Phase 2: edges sorted by dst; each core owns 49 windows of 128 dst nodes.
  Windows are permuted per-core (sorted by slot count) against a shared
  rank-max schedule so the SPMD instruction stream is identical. Per window:
  one dma_gather per bank pulls all K*128 edge rows; onehot machinery is
  built with 4x-mode bf16 tensor_scalar ops; per 128-edge slot: 3 matmuls
  (agg [128,384], den [128,3], s_dst-broadcast [128,3]) accumulate in PSUM.
  Flush: reciprocal of den, per-head projection by wfold, ACT-scaled sum.
"""

import numpy as np
import ml_dtypes

import concourse.bass as bass
import concourse.bacc as bacc
import concourse.mybir as mybir
from concourse.tile import TileContext
from concourse import bass_utils

F32 = mybir.dt.float32
BF16 = mybir.dt.bfloat16
I32 = mybir.dt.int32
I16 = mybir.dt.int16
BF_NP = ml_dtypes.bfloat16

N_NODES = 50000
N_EDGES = 600000
DIM = 128
NH = 3
NEG = 0.2

NCORES = 8
NPW = 128                    # dst nodes per window
WPC = 49                     # windows per core
NPC = NPW * WPC              # 6272
NTILES = 392                 # node tiles of 128 (50176 nodes incl pad)
NROWS = NTILES * 128         # 50176
ABANK = NROWS // 2           # nodes 0..25087 -> bank A
AROWS = ABANK + 1            # poison row + 25088 node rows
XR = 2 * AROWS               # 50178 total XROW rows
ROWC = 256                   # bf16 cols per row (512B)
SLABS = 8
TPS = NTILES // SLABS        # 49 tiles per slab
KMAX_TILE = 32               # dstcT tile partition allocation

_cache = {}


def _row_of(n):
    """XROW row of node n (vectorized)."""
    n = np.asarray(n)
    return np.where(n < ABANK, 1 + n, AROWS + 1 + (n - ABANK)).astype(np.int64)


def _build(sched, has_obias, has_bmsg, kmax):
    KAs, KBs = sched
    Ks = [a + b for a, b in zip(KAs, KBs)]
    off8 = np.concatenate([[0], np.cumsum([8 * k for k in Ks])]).astype(int)
    off2 = np.concatenate([[0], np.cumsum([2 * k for k in Ks])]).astype(int)
    off128 = np.concatenate([[0], np.cumsum([128 * k for k in Ks])]).astype(int)

    nc = bacc.Bacc("TRN2", target_bir_lowering=False, debug=False,
                   num_devices=NCORES)

    xrow = nc.dram_tensor("xrow", [XR, ROWC], BF16, kind="ExternalInput")
    xt = nc.dram_tensor("xt", [128, NROWS], BF16, kind="ExternalInput")
    asd6 = nc.dram_tensor("asd6", [128, 6], BF16, kind="ExternalInput")
    biasr42 = nc.dram_tensor("biasr42", [128, 42], BF16, kind="ExternalInput")
    wfold = nc.dram_tensor("wfold", [128, NH * DIM], BF16,
                           kind="ExternalInput")
    iota = nc.dram_tensor("iota", [128, 128], BF16, kind="ExternalInput")
    obias = nc.dram_tensor("obias", [128, DIM], F32, kind="ExternalInput")
    bmsg3 = nc.dram_tensor("bmsg3", [NH, DIM], F32, kind="ExternalInput")
    idx_all = nc.dram_tensor("idx_all", [128, off8[-1]], I16,
                             kind="ExternalInput")
    meta_all = nc.dram_tensor("meta_all", [128, off2[-1]], F32,
                              kind="ExternalInput")
    dstcF_all = nc.dram_tensor("dstcF_all", [1, off128[-1]], BF16,
                               kind="ExternalInput")
    iotacol = nc.dram_tensor("iotacol", [128, 1], F32, kind="ExternalInput")
    wnid = nc.dram_tensor("wnid", [NPC, 1], I32, kind="ExternalInput")
    outc = nc.dram_tensor("outc", [NPC, DIM], F32, kind="ExternalOutput")

    with TileContext(nc) as tc:
        with tc.tile_pool(name="const", bufs=1) as cpool:
            asd_sb = cpool.tile([128, 6], BF16, tag="asd")
            nc.sync.dma_start(out=asd_sb[:], in_=asd6[:])
            biasr_sb = cpool.tile([128, 42], BF16, tag="biasr")
            nc.sync.dma_start(out=biasr_sb[:], in_=biasr42[:])
            wf_sb = cpool.tile([128, NH * DIM], BF16, tag="wf")
            nc.sync.dma_start(out=wf_sb[:], in_=wfold[:])
            iota_sb = cpool.tile([128, 128], BF16, tag="iota")
            nc.sync.dma_start(out=iota_sb[:], in_=iota[:])
            iotacol_sb = cpool.tile([128, 1], F32, tag="iotacol")
            nc.sync.dma_start(out=iotacol_sb[:], in_=iotacol[:])
            obias_sb = cpool.tile([128, DIM], F32, tag="obias")
            if has_obias:
                nc.sync.dma_start(out=obias_sb[:], in_=obias[:])
            bmsg_sb = cpool.tile([NH, DIM], F32, tag="bmsg")
            if has_bmsg:
                nc.sync.dma_start(out=bmsg_sb[:], in_=bmsg3[:])

            # ---------------- phase 1: scores ----------------
            with (
                tc.tile_pool(name="p1", bufs=2) as p1,
                tc.tile_pool(name="p1s", bufs=3) as p1s,
                tc.tile_pool(name="p1ps", bufs=2, space="PSUM") as p1ps,
            ):
                for s in range(SLABS):
                    slab = p1.tile([128, TPS * 128], BF16, tag="slab")
                    nc.sync.dma_start(
                        out=slab[:], in_=xt[:, s * TPS * 128:(s + 1) * TPS * 128])
                    scr = p1s.tile([128, TPS * 6], BF16, tag="scr")
                    for g in range(7):          # 7 groups of 7 tiles
                        ps = p1ps.tile([128, 42], F32, tag="ps")
                        for t in range(7):
                            ti = g * 7 + t
                            nc.tensor.matmul(
                                out=ps[:, 6 * t:6 * t + 6],
                                lhsT=slab[:, ti * 128:(ti + 1) * 128],
                                rhs=asd_sb[:], start=True, stop=True)
                        nc.vector.tensor_tensor(
                            out=scr[:, g * 42:(g + 1) * 42], in0=ps[:],
                            in1=biasr_sb[:], op=mybir.AluOpType.add)
                    row0 = (1 + s * TPS * 128 if s < SLABS // 2
                            else AROWS + 1 + (s - SLABS // 2) * TPS * 128)
                    dst_ap = xrow[row0:row0 + TPS * 128, 128:134].rearrange(
                        "(t p) c -> p t c", p=128)
                    src_ap = scr[:].rearrange("p (t c) -> p t c", c=6)
                    nc.sync.dma_start(out=dst_ap, in_=src_ap)

            # ---------------- phase 2: windows ----------------
            with (
                tc.tile_pool(name="gt", bufs=2) as gpool,
                tc.tile_pool(name="win", bufs=3) as wpool,
                tc.tile_pool(name="ot", bufs=3) as otpool,
                tc.tile_pool(name="sm", bufs=3) as smpool,
                tc.tile_pool(name="mst", bufs=4) as mstpool,
                tc.tile_pool(name="fl", bufs=2) as flpool,
                tc.tile_pool(name="stg", bufs=2) as stgpool,
                tc.tile_pool(name="aggps", bufs=2, space="PSUM") as aggps_p,
                tc.tile_pool(name="denps", bufs=2, space="PSUM") as denps_p,
                tc.tile_pool(name="sdps", bufs=2, space="PSUM") as sdps_p,
                tc.tile_pool(name="ops", bufs=2, space="PSUM") as ops_p,
            ):
                stg = None
                for j in range(WPC):
                    KA, KB = KAs[j], KBs[j]
                    K = KA + KB
                    meta = wpool.tile([128, 2 * kmax], F32, tag="meta")
                    nc.sync.dma_start(
                        out=meta[:, 0:2 * K],
                        in_=meta_all[:, off2[j]:off2[j] + 2 * K])
                    idxt = wpool.tile([128, 8 * kmax], I16, tag="idx")
                    nc.sync.dma_start(
                        out=idxt[:, 0:8 * K],
                        in_=idx_all[:, off8[j]:off8[j] + 8 * K])
                    dstcB = wpool.tile([128, kmax * 128], BF16, tag="dstcB")
                    nc.sync.dma_start(
                        out=dstcB[:, 0:K * 128],
                        in_=dstcF_all[0:1, off128[j]:off128[j] + K * 128
                                      ].broadcast_to((128, K * 128)))
                    wn = wpool.tile([128, 1], I32, tag="wn")
                    nc.sync.dma_start(out=wn[:], in_=wnid[j * 128:(j + 1) * 128, :])
                    sdw = wpool.tile([128, ROWC], BF16, tag="sdw")
                    nc.gpsimd.indirect_dma_start(
                        out=sdw[:], out_offset=None, in_=xrow[:],
                        in_offset=bass.IndirectOffsetOnAxis(ap=wn[:, 0:1], axis=0))

                    gt = gpool.tile([128, kmax * ROWC], BF16, tag="gt")
                    gv = gt[:, 0:K * ROWC].rearrange("p (k c) -> p k c", c=ROWC)
                    if KA > 0:
                        nc.gpsimd.dma_gather(
                            gv[:, 0:KA, :], xrow[0:AROWS, :],
                            idxt[:, 0:8 * KA], KA * 128, KA * 128, ROWC)
                    if KB > 0:
                        nc.gpsimd.dma_gather(
                            gv[:, KA:K, :], xrow[AROWS:XR, :],
                            idxt[:, 8 * KA:8 * K], KB * 128, KB * 128, ROWC)

                    # s_dst per edge: oT[d,e] = (dstcT[k,e]==d); sd = oT^T @ sdw
                    sdps = sdps_p.tile([128, 3 * kmax], F32, tag="sdps")
                    for k in range(K):
                        oT = otpool.tile([128, 128], BF16, tag="oT")
                        nc.vector.tensor_scalar(
                            out=oT[:], in0=dstcB[:, k * 128:(k + 1) * 128],
                            scalar1=iotacol_sb[:, 0:1], scalar2=None,
                            op0=mybir.AluOpType.is_equal)
                        nc.tensor.matmul(
                            out=sdps[:, 3 * k:3 * k + 3], lhsT=oT[:],
                            rhs=sdw[:, 131:134], start=True, stop=True)

                    # scores: e = lrelu(s_src + s_dst); p = exp(e); q = p*ew
                    e1 = smpool.tile([128, 3 * kmax], BF16, tag="e1")
                    nc.vector.tensor_tensor(
                        out=e1[:, 0:3 * K].rearrange("p (k c) -> p k c", c=3),
                        in0=gv[:, :, 128:131],
                        in1=sdps[:, 0:3 * K].rearrange("p (k c) -> p k c", c=3),
                        op=mybir.AluOpType.add)
                    sl = smpool.tile([128, 3 * kmax], BF16, tag="sl")
                    nc.vector.tensor_scalar(
                        out=sl[:, 0:3 * K], in0=e1[:, 0:3 * K], scalar1=NEG,
                        scalar2=None, op0=mybir.AluOpType.mult)
                    e2 = smpool.tile([128, 3 * kmax], BF16, tag="e2")
                    nc.vector.tensor_tensor(
                        out=e2[:, 0:3 * K], in0=e1[:, 0:3 * K],
                        in1=sl[:, 0:3 * K], op=mybir.AluOpType.max)
                    p_all = smpool.tile([128, 3 * kmax], BF16, tag="p")
                    nc.scalar.activation(
                        out=p_all[:, 0:3 * K], in_=e2[:, 0:3 * K],
                        func=mybir.ActivationFunctionType.Exp)
                    q_all = smpool.tile([128, 3 * kmax], F32, tag="q")
                    nc.vector.tensor_tensor(
                        out=q_all[:, 0:3 * K].rearrange("p (k c) -> p k c", c=3),
                        in0=p_all[:, 0:3 * K].rearrange("p (k c) -> p k c", c=3),
                        in1=meta[:, K:2 * K].unsqueeze(2).broadcast_to(
                            (128, K, 3)),
                        op=mybir.AluOpType.mult)

                    aggps = aggps_p.tile([128, NH * 128], F32, tag="agg")
                    denps = denps_p.tile([128, 8], F32, tag="den")
                    for k in range(K):
                        mst = mstpool.tile([128, 512], BF16, tag="mst")
                        dcol = meta[:, k:k + 1]
                        nc.vector.tensor_scalar(
                            out=mst[:, 384:512], in0=iota_sb[:],
                            scalar1=dcol, scalar2=None,
                            op0=mybir.AluOpType.is_equal)
                        for h in range(2):
                            nc.vector.tensor_scalar(
                                out=mst[:, h * 128:(h + 1) * 128],
                                in0=iota_sb[:], scalar1=dcol,
                                scalar2=q_all[:, 3 * k + h:3 * k + h + 1],
                                op0=mybir.AluOpType.is_equal,
                                op1=mybir.AluOpType.mult)
                        nc.scalar.activation(
                            out=mst[:, 256:384], in_=mst[:, 384:512],
                            func=mybir.ActivationFunctionType.Copy,
                            scale=q_all[:, 3 * k + 2:3 * k + 3])
                        nc.tensor.matmul(
                            out=aggps[:], lhsT=gt[:, k * ROWC:k * ROWC + 128],
                            rhs=mst[:, 0:384], start=(k == 0), stop=(k == K - 1))
                        nc.tensor.matmul(
                            out=denps[:, 0:3], lhsT=mst[:, 384:512],
                            rhs=p_all[:, 3 * k:3 * k + 3],
                            start=(k == 0), stop=(k == K - 1))
                        if has_bmsg:
                            nc.tensor.matmul(
                                out=denps[:, 3:6], lhsT=mst[:, 384:512],
                                rhs=q_all[:, 3 * k:3 * k + 3],
                                start=(k == 0), stop=(k == K - 1))

                    # ---- window flush ----
                    dinv = flpool.tile([128, 3], F32, tag="dinv")
                    nc.vector.tensor_scalar(
                        out=dinv[:], in0=denps[:, 0:3], scalar1=1e-16,
                        scalar2=None, op0=mybir.AluOpType.max)
                    nc.vector.reciprocal(out=dinv[:], in_=dinv[:])
                    aggsb = flpool.tile([128, NH * 128], BF16, tag="aggsb")
                    nc.scalar.activation(
                        out=aggsb[:], in_=aggps[:],
                        func=mybir.ActivationFunctionType.Copy)
                    ths = []
                    for h in range(NH):
                        o2 = ops_p.tile([128, DIM], F32, tag="o2")
                        nc.tensor.matmul(
                            out=o2[:], lhsT=aggsb[:, h * 128:(h + 1) * 128],
                            rhs=wf_sb[:, h * DIM:(h + 1) * DIM],
                            start=True, stop=True)
                        th = flpool.tile([128, DIM], F32, tag=f"th{h}")
                        nc.scalar.activation(
                            out=th[:], in_=o2[:],
                            func=mybir.ActivationFunctionType.Copy,
                            scale=dinv[:, h:h + 1])
                        ths.append(th)
                    if j % 7 == 0:
                        stg = stgpool.tile([128, 7 * DIM], F32, tag="stg")
                    sg = stg[:, (j % 7) * DIM:(j % 7 + 1) * DIM]
                    a01 = flpool.tile([128, DIM], F32, tag="a01")
                    nc.vector.tensor_tensor(out=a01[:], in0=ths[0][:],
                                            in1=ths[1][:],
                                            op=mybir.AluOpType.add)
                    if has_bmsg:
                        a01b = flpool.tile([128, DIM], F32, tag="a01b")
                        nc.vector.tensor_tensor(out=a01b[:], in0=a01[:],
                                                in1=ths[2][:],
                                                op=mybir.AluOpType.add)
                        tn = flpool.tile([128, 3], F32, tag="tn")
                        nc.vector.tensor_tensor(out=tn[:], in0=denps[:, 3:6],
                                                in1=dinv[:],
                                                op=mybir.AluOpType.mult)
                        bc = flpool.tile([128, DIM], F32, tag="bc")
                        for h in range(NH):
                            dst_t = bc if h == 0 else flpool.tile(
                                [128, DIM], F32, tag="bch")
                            nc.vector.tensor_scalar(
                                out=dst_t[:],
                                in0=bmsg_sb[h:h + 1, :].partition_broadcast(128),
                                scalar1=tn[:, h:h + 1], scalar2=None,
                                op0=mybir.AluOpType.mult)
                            if h > 0:
                                nc.vector.tensor_tensor(
                                    out=bc[:], in0=bc[:], in1=dst_t[:],
                                    op=mybir.AluOpType.add)
                        if has_obias:
                            nc.vector.tensor_tensor(
                                out=bc[:], in0=bc[:], in1=obias_sb[:],
                                op=mybir.AluOpType.add)
                        nc.vector.tensor_tensor(out=sg, in0=a01b[:],
                                                in1=bc[:],
                                                op=mybir.AluOpType.add)
                    else:
                        if has_obias:
                            a2 = flpool.tile([128, DIM], F32, tag="a2")
                            nc.vector.tensor_tensor(out=a2[:], in0=ths[2][:],
                                                    in1=obias_sb[:],
                                                    op=mybir.AluOpType.add)
                            nc.vector.tensor_tensor(out=sg, in0=a01[:],
                                                    in1=a2[:],
                                                    op=mybir.AluOpType.add)
                        else:
                            nc.vector.tensor_tensor(out=sg, in0=a01[:],
                                                    in1=ths[2][:],
                                                    op=mybir.AluOpType.add)
                    if j % 7 == 6:
                        j0 = j - 6
                        dst_ap = outc[j0 * 128:(j0 + 7) * 128, :].rearrange(
                            "(g p) c -> p g c", p=128)
                        nc.sync.dma_start(
                            out=dst_ap,
                            in_=stg[:].rearrange("p (g c) -> p g c", c=DIM))

    nc.compile()
    return nc


def _prep(x, edge_index, edge_ids, ddi_weight, W_lin, b_lin, edge_emb,
          W_heads, att_src, att_dst, bias_heads):
    x = np.asarray(x, np.float32)
    src = np.asarray(edge_index[0], np.int64)
    dst = np.asarray(edge_index[1], np.int64)
    eids = np.asarray(edge_ids, np.int64)
    ddi = np.asarray(ddi_weight, np.float32)
    W_lin = np.asarray(W_lin, np.float32)
    b_lin = np.asarray(b_lin, np.float32)
    edge_emb = np.asarray(edge_emb, np.float32)
    W_heads = np.asarray(W_heads, np.float32)
    att_src = np.asarray(att_src, np.float32)
    att_dst = np.asarray(att_dst, np.float32)
    bias_heads = np.asarray(bias_heads, np.float32)

    # ---- folded weights ----
    asd = np.zeros((DIM, 6), np.float32)
    sbias = np.zeros(6, np.float32)
    for h in range(NH):
        wsrc = W_lin @ W_heads[h] @ att_src[h]
        wdst = W_lin @ W_heads[h] @ att_dst[h]
        asd[:, h] = wsrc
        asd[:, 3 + h] = wdst
        sbias[h] = b_lin @ W_heads[h] @ att_src[h]
        sbias[3 + h] = b_lin @ W_heads[h] @ att_dst[h]
    wfold = np.zeros((DIM, NH * DIM), np.float32)
    bmsg3 = np.zeros((NH, DIM), np.float32)
    for h in range(NH):
        wfold[:, h * DIM:(h + 1) * DIM] = (W_lin @ W_heads[h]) / NH
        bmsg3[h] = (b_lin @ W_heads[h]) / NH
    obias = np.tile(bias_heads.sum(0) / NH, (128, 1)).astype(np.float32)
    has_obias = bool(np.abs(obias).max() > 0)
    has_bmsg = bool(np.abs(bmsg3).max() > 0)

    # ---- node rows ----
    xb = np.zeros((NROWS, DIM), BF_NP)
    xb[:N_NODES] = x.astype(BF_NP)
    xrow = np.zeros((XR, ROWC), BF_NP)
    xrow[1:AROWS, 0:DIM] = xb[0:ABANK]
    xrow[AROWS + 1:XR, 0:DIM] = xb[ABANK:]
    xt = np.ascontiguousarray(xb.T).astype(BF_NP)        # [128, NROWS]

    # ---- edges: sort by dst, split per window by src bank ----
    order = np.argsort(dst, kind="stable")
    src_s = src[order].astype(np.int64)
    dst_s = dst[order].astype(np.int64)
    ew_s = (edge_emb[eids[order], 0] - ddi[order]).astype(np.float32)
    bounds = np.searchsorted(dst_s, np.arange(0, NROWS + NPW, NPW))

    WN = NCORES * WPC
    winA, winB = [], []
    for wi in range(WN):
        e0, e1 = int(bounds[wi]), int(bounds[wi + 1])
        s_, d_, w_ = src_s[e0:e1], dst_s[e0:e1], ew_s[e0:e1]
        am = s_ < ABANK
        winA.append((s_[am], d_[am], w_[am]))
        winB.append((s_[~am], d_[~am], w_[~am]))

    def kcnt(n):
        return (n + 127) // 128

    # snake-deal windows to cores by descending slot count so each core's
    # rank-j window has a near-identical K, minimizing rank-max padding
    kw = [kcnt(len(winA[g][0])) + kcnt(len(winB[g][0])) for g in range(WN)]
    order_w = np.argsort(-np.asarray(kw), kind="stable")
    asgn = [[] for _ in range(NCORES)]
    for r in range(WPC):
        cores = range(NCORES) if r % 2 == 0 else range(NCORES - 1, -1, -1)
        for i, c in enumerate(cores):
            asgn[c].append(int(order_w[r * NCORES + i]))
    for c in range(NCORES):
        asgn[c].sort(key=lambda g: -kw[g])
    KAs, KBs = [], []
    for j in range(WPC):
        KAs.append(max(kcnt(len(winA[asgn[c][j]][0]))
                       for c in range(NCORES)))
        KBs.append(max(kcnt(len(winB[asgn[c][j]][0]))
                       for c in range(NCORES)))
    kmax = max(a + b for a, b in zip(KAs, KBs))
    sched = (tuple(KAs), tuple(KBs))
    Ks = [a + b for a, b in zip(KAs, KBs)]
    off8 = np.concatenate([[0], np.cumsum([8 * k for k in Ks])]).astype(int)
    off2 = np.concatenate([[0], np.cumsum([2 * k for k in Ks])]).astype(int)
    KTOT = int(sum(Ks))
    assert kmax <= KMAX_TILE

    iota = np.tile(np.arange(128, dtype=np.float32), (128, 1)).astype(BF_NP)
    iotacol = np.arange(128, dtype=np.float32)[:, None]
    off128 = np.concatenate([[0], np.cumsum([128 * k for k in Ks])]).astype(int)
    biasr42 = np.tile(sbias, (128, 7)).astype(BF_NP)

    shared = dict(
        xrow=xrow, xt=xt, asd6=asd.astype(BF_NP), biasr42=biasr42,
        wfold=wfold.astype(BF_NP), iota=iota, iotacol=iotacol, obias=obias,
        bmsg3=bmsg3)

    in_maps = []
    perm_list = []
    for c in range(NCORES):
        idx_all = np.zeros((128, off8[-1]), np.int16)
        meta_all = np.zeros((128, off2[-1]), np.float32)
        dstcF_all = np.full((1, off128[-1]), 128.0, BF_NP)
        wnid_a = np.zeros((NPC, 1), np.int32)
        for j in range(WPC):
            gw = asgn[c][j]
            KA, KB, K = KAs[j], KBs[j], Ks[j]
            base = gw * NPW
            dstc = np.full((128, K), 128.0, np.float32)
            ewn = np.zeros((128, K), np.float32)
            idxs = np.zeros(8 * K * 16, np.int64).reshape(16, 8 * K)

            for (s_, d_, w_), k0, rowoff in (
                    (winA[gw], 0, 1),
                    (winB[gw], KA, 1 - ABANK)):
                n = len(s_)
                if n == 0:
                    continue
                i = np.arange(n)
                p = i % 128
                k = k0 + i // 128
                dstc[p, k] = (d_ - base).astype(np.float32)
                ewn[p, k] = w_
                rows = (s_ + rowoff).astype(np.int64)   # bank-local row id
                # gather-local position i -> idx slot [i%16, 8*k0*2 + i//16]
                idxs[i % 16, 8 * k0 + i // 16] = rows
            # Q7 gather reads the idx stream from one 16-partition group
            # (which one depends on the queue) — replicate to all 8 groups.
            idx_all[:, off8[j]:off8[j + 1]] = np.tile(
                idxs.astype(np.int16), (8, 1))
            meta_all[:, off2[j]:off2[j] + K] = dstc
            meta_all[:, off2[j] + K:off2[j] + 2 * K] = ewn
            dstcF_all[0, off128[j]:off128[j] + K * 128] = \
                dstc.T.reshape(-1).astype(BF_NP)
            wnid_a[j * 128:(j + 1) * 128, 0] = _row_of(
                np.arange(base, base + NPW))
        m = dict(shared)
        m.update(idx_all=idx_all, meta_all=meta_all, dstcF_all=dstcF_all,
                 wnid=wnid_a)
        in_maps.append(m)
        perm_list.append(asgn[c])

    key = (sched, has_obias, has_bmsg)
    return key, in_maps, perm_list


def kernel(**inputs):
    key, in_maps, perm_list = _prep(**inputs)
    sched, has_obias, has_bmsg = key
    kmax = max(a + b for a, b in zip(sched[0], sched[1]))
    if key not in _cache:
        _cache[key] = _build(sched, has_obias, has_bmsg, kmax)
    nc = _cache[key]
    res = bass_utils.run_bass_kernel_spmd(nc, in_maps,
                                          core_ids=list(range(NCORES)))
    out = np.zeros((NROWS, DIM), np.float32)
    for c in range(NCORES):
        oc = res.results[c]["outc"]
        for j in range(WPC):
            gw = int(perm_list[c][j])
            out[gw * NPW:(gw + 1) * NPW] = oc[j * 128:(j + 1) * 128]
    return np.ascontiguousarray(out[:N_NODES]).astype(np.float32)


# revision 21
# speedup vs baseline: 4.7914x; 1.1414x over previous
"""GAT (3-head, edge-weighted) message-passing kernel for 8 Trainium2 NeuronCores.

v2 design (bf16 + dma_gather + folded weights):
  Host folds W_lin into everything: scores s = x @ (W_lin W_h a), messages
  aggregate RAW x and project by W' = W_lin W_h / 3 at the end. Node rows
  live in XROW [50178, 256] bf16 = [x(128) | s_src(3) | s_dst(3) | pad],
  512B rows (dma_gather elem multiple of 256B). Rows are split into two
  banks of <=25089 rows with a poison row 0 each so gather indices fit in
  int16; pad slots index the all-zero poison row.

  Phase 1 (per core, replicated): 8 slab loads of xT bf16, 392 tiny matmuls
  x_tile @ asd6 -> 6 score cols, written into XROW score columns.

  Phase 2: edges sorted by dst; each core owns 49 windows of 128 dst nodes.
  Windows are permuted per-core (sorted by slot count) against a shared
  rank-max schedule so the SPMD instruction stream is identical. Per window:
  one dma_gather per bank pulls all K*128 edge rows; onehot machinery is
  built with 4x-mode bf16 tensor_scalar ops; per 128-edge slot: 3 matmuls
  (agg [128,384], den [128,3], s_dst-broadcast [128,3]) accumulate in PSUM.
  Flush: reciprocal of den, per-head projection by wfold, ACT-scaled sum.
"""

import numpy as np
import ml_dtypes

import concourse.bass as bass
import concourse.bacc as bacc
import concourse.mybir as mybir
from concourse.tile import TileContext
from concourse import bass_utils

F32 = mybir.dt.float32
FP8 = mybir.dt.float8e4
BF16 = mybir.dt.bfloat16
I32 = mybir.dt.int32
I16 = mybir.dt.int16
BF_NP = ml_dtypes.bfloat16
E4_NP = ml_dtypes.float8_e4m3

N_NODES = 50000
N_EDGES = 600000
DIM = 128
NH = 3
NEG = 0.2

NCORES = 8
NPW = 128                    # dst nodes per window
WPC = 49                     # windows per core
NPC = NPW * WPC              # 6272
NTILES = 392                 # node tiles of 128 (50176 nodes incl pad)
NROWS = NTILES * 128         # 50176
ABANK = NROWS // 2           # nodes 0..25087 -> bank A
AROWS = ABANK + 1            # poison row + 25088 node rows
XR = 2 * AROWS               # 50178 total XROW rows
ROWC = 256                   # bf16 cols per row (512B)
SLABS = 8
TPS = NTILES // SLABS        # 49 tiles per slab
KMAX_TILE = 32               # dstcT tile partition allocation

_cache = {}


def _row_of(n):
    """XROW row of node n (vectorized)."""
    n = np.asarray(n)
    return np.where(n < ABANK, 1 + n, AROWS + 1 + (n - ABANK)).astype(np.int64)


def _build(sched, has_obias, has_bmsg, kmax):
    KAs, KBs = sched
    Ks = [a + b for a, b in zip(KAs, KBs)]
    off8 = np.concatenate([[0], np.cumsum([8 * k for k in Ks])]).astype(int)
    off2 = np.concatenate([[0], np.cumsum([2 * k for k in Ks])]).astype(int)
    off128 = np.concatenate([[0], np.cumsum([128 * k for k in Ks])]).astype(int)

    nc = bacc.Bacc("TRN2", target_bir_lowering=False, debug=False,
                   num_devices=NCORES)

    xrow = nc.dram_tensor("xrow", [XR, ROWC], BF16, kind="ExternalInput")
    xt = nc.dram_tensor("xt", [128, NROWS], BF16, kind="ExternalInput")
    asd6 = nc.dram_tensor("asd6", [128, 6], BF16, kind="ExternalInput")
    biasr42 = nc.dram_tensor("biasr42", [128, 42], BF16, kind="ExternalInput")
    wfold = nc.dram_tensor("wfold", [128, NH * DIM], BF16,
                           kind="ExternalInput")
    iota = nc.dram_tensor("iota", [128, 128], BF16, kind="ExternalInput")
    obias = nc.dram_tensor("obias", [128, DIM], F32, kind="ExternalInput")
    bmsg3 = nc.dram_tensor("bmsg3", [NH, DIM], F32, kind="ExternalInput")
    idx_all = nc.dram_tensor("idx_all", [128, off8[-1]], I16,
                             kind="ExternalInput")
    meta_all = nc.dram_tensor("meta_all", [128, off2[-1]], F32,
                              kind="ExternalInput")
    oth_all = nc.dram_tensor("oth_all", [128, off128[-1]], FP8,
                             kind="ExternalInput")
    wnid = nc.dram_tensor("wnid", [NPC, 1], I32, kind="ExternalInput")
    outc = nc.dram_tensor("outc", [NPC, DIM], F32, kind="ExternalOutput")

    with TileContext(nc) as tc:
        with tc.tile_pool(name="const", bufs=1) as cpool:
            asd_sb = cpool.tile([128, 6], BF16, tag="asd")
            nc.sync.dma_start(out=asd_sb[:], in_=asd6[:])
            biasr_sb = cpool.tile([128, 42], BF16, tag="biasr")
            nc.sync.dma_start(out=biasr_sb[:], in_=biasr42[:])
            wf_sb = cpool.tile([128, NH * DIM], BF16, tag="wf")
            nc.sync.dma_start(out=wf_sb[:], in_=wfold[:])
            iota_sb = cpool.tile([128, 128], BF16, tag="iota")
            nc.sync.dma_start(out=iota_sb[:], in_=iota[:])
            obias_sb = cpool.tile([128, DIM], F32, tag="obias")
            if has_obias:
                nc.sync.dma_start(out=obias_sb[:], in_=obias[:])
            bmsg_sb = cpool.tile([NH, DIM], F32, tag="bmsg")
            if has_bmsg:
                nc.sync.dma_start(out=bmsg_sb[:], in_=bmsg3[:])

            # ---------------- phase 1: scores ----------------
            with (
                tc.tile_pool(name="p1", bufs=2) as p1,
                tc.tile_pool(name="p1s", bufs=3) as p1s,
                tc.tile_pool(name="p1ps", bufs=2, space="PSUM") as p1ps,
            ):
                for s in range(SLABS):
                    slab = p1.tile([128, TPS * 128], BF16, tag="slab")
                    nc.sync.dma_start(
                        out=slab[:], in_=xt[:, s * TPS * 128:(s + 1) * TPS * 128])
                    scr = p1s.tile([128, TPS * 6], BF16, tag="scr")
                    for g in range(7):          # 7 groups of 7 tiles
                        ps = p1ps.tile([128, 42], F32, tag="ps")
                        for t in range(7):
                            ti = g * 7 + t
                            nc.tensor.matmul(
                                out=ps[:, 6 * t:6 * t + 6],
                                lhsT=slab[:, ti * 128:(ti + 1) * 128],
                                rhs=asd_sb[:], start=True, stop=True)
                        nc.vector.tensor_tensor(
                            out=scr[:, g * 42:(g + 1) * 42], in0=ps[:],
                            in1=biasr_sb[:], op=mybir.AluOpType.add)
                    row0 = (1 + s * TPS * 128 if s < SLABS // 2
                            else AROWS + 1 + (s - SLABS // 2) * TPS * 128)
                    dst_ap = xrow[row0:row0 + TPS * 128, 128:134].rearrange(
                        "(t p) c -> p t c", p=128)
                    src_ap = scr[:].rearrange("p (t c) -> p t c", c=6)
                    nc.sync.dma_start(out=dst_ap, in_=src_ap)

            # ---------------- phase 2: windows ----------------
            with (
                tc.tile_pool(name="gt", bufs=4) as gpool,
                tc.tile_pool(name="win", bufs=6) as wpool,
                tc.tile_pool(name="ot", bufs=6) as otpool,
                tc.tile_pool(name="sm", bufs=6) as smpool,
                tc.tile_pool(name="mst", bufs=8) as mstpool,
                tc.tile_pool(name="fl", bufs=4) as flpool,
                tc.tile_pool(name="stg", bufs=2) as stgpool,
                tc.tile_pool(name="aggps", bufs=2, space="PSUM") as aggps_p,
                tc.tile_pool(name="denps", bufs=2, space="PSUM") as denps_p,
                tc.tile_pool(name="sdps", bufs=2, space="PSUM") as sdps_p,
                tc.tile_pool(name="ops", bufs=2, space="PSUM") as ops_p,
            ):
                stg = None
                for j in range(WPC):
                    KA, KB = KAs[j], KBs[j]
                    K = KA + KB
                    meta = wpool.tile([128, 2 * kmax], F32, tag="meta")
                    nc.sync.dma_start(
                        out=meta[:, 0:2 * K],
                        in_=meta_all[:, off2[j]:off2[j] + 2 * K])
                    idxt = wpool.tile([128, 8 * kmax], I16, tag="idx")
                    nc.sync.dma_start(
                        out=idxt[:, 0:8 * K],
                        in_=idx_all[:, off8[j]:off8[j] + 8 * K])
                    oth = wpool.tile([128, kmax * 128], FP8, tag="oth")
                    nc.sync.dma_start(
                        out=oth[:, 0:K * 128],
                        in_=oth_all[:, off128[j]:off128[j] + K * 128])
                    wn = wpool.tile([128, 1], I32, tag="wn")
                    nc.sync.dma_start(out=wn[:], in_=wnid[j * 128:(j + 1) * 128, :])
                    sdw = wpool.tile([128, ROWC], BF16, tag="sdw")
                    nc.gpsimd.indirect_dma_start(
                        out=sdw[:], out_offset=None, in_=xrow[:],
                        in_offset=bass.IndirectOffsetOnAxis(ap=wn[:, 0:1], axis=0))

                    gt = gpool.tile([128, kmax * ROWC], BF16, tag="gt")
                    gv = gt[:, 0:K * ROWC].rearrange("p (k c) -> p k c", c=ROWC)
                    if KA > 0:
                        nc.gpsimd.dma_gather(
                            gv[:, 0:KA, :], xrow[0:AROWS, :],
                            idxt[:, 0:8 * KA], KA * 128, KA * 128, ROWC)
                    if KB > 0:
                        nc.gpsimd.dma_gather(
                            gv[:, KA:K, :], xrow[AROWS:XR, :],
                            idxt[:, 8 * KA:8 * K], KB * 128, KB * 128, ROWC)

                    # s_dst per edge: oT[d,e] = (dstcT[k,e]==d); sd = oT^T @ sdw
                    sdps = sdps_p.tile([128, 3 * kmax], F32, tag="sdps")
                    for k in range(K):
                        nc.tensor.matmul(
                            out=sdps[:, 3 * k:3 * k + 3],
                            lhsT=oth[:, k * 128:(k + 1) * 128],
                            rhs=sdw[:, 131:134], start=True, stop=True)

                    # scores: e = lrelu(s_src + s_dst); p = exp(e); q = p*ew
                    e1 = smpool.tile([128, 3 * kmax], BF16, tag="e1")
                    nc.vector.tensor_tensor(
                        out=e1[:, 0:3 * K].rearrange("p (k c) -> p k c", c=3),
                        in0=gv[:, :, 128:131],
                        in1=sdps[:, 0:3 * K].rearrange("p (k c) -> p k c", c=3),
                        op=mybir.AluOpType.add)
                    sl = smpool.tile([128, 3 * kmax], BF16, tag="sl")
                    nc.vector.tensor_scalar(
                        out=sl[:, 0:3 * K], in0=e1[:, 0:3 * K], scalar1=NEG,
                        scalar2=None, op0=mybir.AluOpType.mult)
                    e2 = smpool.tile([128, 3 * kmax], BF16, tag="e2")
                    nc.vector.tensor_tensor(
                        out=e2[:, 0:3 * K], in0=e1[:, 0:3 * K],
                        in1=sl[:, 0:3 * K], op=mybir.AluOpType.max)
                    p_all = smpool.tile([128, 3 * kmax], BF16, tag="p")
                    nc.scalar.activation(
                        out=p_all[:, 0:3 * K], in_=e2[:, 0:3 * K],
                        func=mybir.ActivationFunctionType.Exp)
                    q_all = smpool.tile([128, 3 * kmax], F32, tag="q")
                    nc.vector.tensor_tensor(
                        out=q_all[:, 0:3 * K].rearrange("p (k c) -> p k c", c=3),
                        in0=p_all[:, 0:3 * K].rearrange("p (k c) -> p k c", c=3),
                        in1=meta[:, K:2 * K].unsqueeze(2).broadcast_to(
                            (128, K, 3)),
                        op=mybir.AluOpType.mult)

                    aggps = aggps_p.tile([128, NH * 128], F32, tag="agg")
                    denps = denps_p.tile([128, 8], F32, tag="den")
                    for k in range(K):
                        mst = mstpool.tile([128, 512], BF16, tag="mst")
                        dcol = meta[:, k:k + 1]
                        nc.vector.tensor_scalar(
                            out=mst[:, 384:512], in0=iota_sb[:],
                            scalar1=dcol, scalar2=None,
                            op0=mybir.AluOpType.is_equal)
                        for h in range(2):
                            nc.vector.tensor_scalar(
                                out=mst[:, h * 128:(h + 1) * 128],
                                in0=iota_sb[:], scalar1=dcol,
                                scalar2=q_all[:, 3 * k + h:3 * k + h + 1],
                                op0=mybir.AluOpType.is_equal,
                                op1=mybir.AluOpType.mult)
                        if k % 8 == 7:
                            nc.vector.tensor_scalar(
                                out=mst[:, 256:384], in0=iota_sb[:],
                                scalar1=dcol,
                                scalar2=q_all[:, 3 * k + 2:3 * k + 3],
                                op0=mybir.AluOpType.is_equal,
                                op1=mybir.AluOpType.mult)
                        else:
                            nc.scalar.activation(
                                out=mst[:, 256:384], in_=mst[:, 384:512],
                                func=mybir.ActivationFunctionType.Copy,
                                scale=q_all[:, 3 * k + 2:3 * k + 3])
                        nc.tensor.matmul(
                            out=aggps[:], lhsT=gt[:, k * ROWC:k * ROWC + 128],
                            rhs=mst[:, 0:384], start=(k == 0), stop=(k == K - 1))
                        nc.tensor.matmul(
                            out=denps[:, 0:3], lhsT=mst[:, 384:512],
                            rhs=p_all[:, 3 * k:3 * k + 3],
                            start=(k == 0), stop=(k == K - 1))
                        if has_bmsg:
                            nc.tensor.matmul(
                                out=denps[:, 3:6], lhsT=mst[:, 384:512],
                                rhs=q_all[:, 3 * k:3 * k + 3],
                                start=(k == 0), stop=(k == K - 1))

                    # ---- window flush ----
                    dinv = flpool.tile([128, 3], F32, tag="dinv")
                    nc.vector.tensor_scalar(
                        out=dinv[:], in0=denps[:, 0:3], scalar1=1e-16,
                        scalar2=None, op0=mybir.AluOpType.max)
                    nc.vector.reciprocal(out=dinv[:], in_=dinv[:])
                    aggsb = flpool.tile([128, NH * 128], BF16, tag="aggsb")
                    if j % 2 == 0:
                        nc.scalar.activation(
                            out=aggsb[:], in_=aggps[:],
                            func=mybir.ActivationFunctionType.Copy)
                    else:
                        nc.vector.tensor_scalar(
                            out=aggsb[:], in0=aggps[:], scalar1=0.0,
                            scalar2=None, op0=mybir.AluOpType.add)
                    ths = []
                    for h in range(NH):
                        o2 = ops_p.tile([128, DIM], F32, tag="o2")
                        nc.tensor.matmul(
                            out=o2[:], lhsT=aggsb[:, h * 128:(h + 1) * 128],
                            rhs=wf_sb[:, h * DIM:(h + 1) * DIM],
                            start=True, stop=True)
                        th = flpool.tile([128, DIM], F32, tag=f"th{h}")
                        nc.scalar.activation(
                            out=th[:], in_=o2[:],
                            func=mybir.ActivationFunctionType.Copy,
                            scale=dinv[:, h:h + 1])
                        ths.append(th)
                    if j % 7 == 0:
                        stg = stgpool.tile([128, 7 * DIM], F32, tag="stg")
                    sg = stg[:, (j % 7) * DIM:(j % 7 + 1) * DIM]
                    a01 = flpool.tile([128, DIM], F32, tag="a01")
                    nc.vector.tensor_tensor(out=a01[:], in0=ths[0][:],
                                            in1=ths[1][:],
                                            op=mybir.AluOpType.add)
                    if has_bmsg:
                        a01b = flpool.tile([128, DIM], F32, tag="a01b")
                        nc.vector.tensor_tensor(out=a01b[:], in0=a01[:],
                                                in1=ths[2][:],
                                                op=mybir.AluOpType.add)
                        tn = flpool.tile([128, 3], F32, tag="tn")
                        nc.vector.tensor_tensor(out=tn[:], in0=denps[:, 3:6],
                                                in1=dinv[:],
                                                op=mybir.AluOpType.mult)
                        bc = flpool.tile([128, DIM], F32, tag="bc")
                        for h in range(NH):
                            dst_t = bc if h == 0 else flpool.tile(
                                [128, DIM], F32, tag="bch")
                            nc.vector.tensor_scalar(
                                out=dst_t[:],
                                in0=bmsg_sb[h:h + 1, :].partition_broadcast(128),
                                scalar1=tn[:, h:h + 1], scalar2=None,
                                op0=mybir.AluOpType.mult)
                            if h > 0:
                                nc.vector.tensor_tensor(
                                    out=bc[:], in0=bc[:], in1=dst_t[:],
                                    op=mybir.AluOpType.add)
                        if has_obias:
                            nc.vector.tensor_tensor(
                                out=bc[:], in0=bc[:], in1=obias_sb[:],
                                op=mybir.AluOpType.add)
                        nc.vector.tensor_tensor(out=sg, in0=a01b[:],
                                                in1=bc[:],
                                                op=mybir.AluOpType.add)
                    else:
                        if has_obias:
                            a2 = flpool.tile([128, DIM], F32, tag="a2")
                            nc.vector.tensor_tensor(out=a2[:], in0=ths[2][:],
                                                    in1=obias_sb[:],
                                                    op=mybir.AluOpType.add)
                            nc.vector.tensor_tensor(out=sg, in0=a01[:],
                                                    in1=a2[:],
                                                    op=mybir.AluOpType.add)
                        else:
                            nc.vector.tensor_tensor(out=sg, in0=a01[:],
                                                    in1=ths[2][:],
                                                    op=mybir.AluOpType.add)
                    if j % 7 == 6:
                        j0 = j - 6
                        dst_ap = outc[j0 * 128:(j0 + 7) * 128, :].rearrange(
                            "(g p) c -> p g c", p=128)
                        nc.sync.dma_start(
                            out=dst_ap,
                            in_=stg[:].rearrange("p (g c) -> p g c", c=DIM))

    nc.compile()
    return nc


def _prep(x, edge_index, edge_ids, ddi_weight, W_lin, b_lin, edge_emb,
          W_heads, att_src, att_dst, bias_heads):
    x = np.asarray(x, np.float32)
    src = np.asarray(edge_index[0], np.int64)
    dst = np.asarray(edge_index[1], np.int64)
    eids = np.asarray(edge_ids, np.int64)
    ddi = np.asarray(ddi_weight, np.float32)
    W_lin = np.asarray(W_lin, np.float32)
    b_lin = np.asarray(b_lin, np.float32)
    edge_emb = np.asarray(edge_emb, np.float32)
    W_heads = np.asarray(W_heads, np.float32)
    att_src = np.asarray(att_src, np.float32)
    att_dst = np.asarray(att_dst, np.float32)
    bias_heads = np.asarray(bias_heads, np.float32)

    # ---- folded weights ----
    asd = np.zeros((DIM, 6), np.float32)
    sbias = np.zeros(6, np.float32)
    for h in range(NH):
        wsrc = W_lin @ W_heads[h] @ att_src[h]
        wdst = W_lin @ W_heads[h] @ att_dst[h]
        asd[:, h] = wsrc
        asd[:, 3 + h] = wdst
        sbias[h] = b_lin @ W_heads[h] @ att_src[h]
        sbias[3 + h] = b_lin @ W_heads[h] @ att_dst[h]
    wfold = np.zeros((DIM, NH * DIM), np.float32)
    bmsg3 = np.zeros((NH, DIM), np.float32)
    for h in range(NH):
        wfold[:, h * DIM:(h + 1) * DIM] = (W_lin @ W_heads[h]) / NH
        bmsg3[h] = (b_lin @ W_heads[h]) / NH
    obias = np.tile(bias_heads.sum(0) / NH, (128, 1)).astype(np.float32)
    has_obias = bool(np.abs(obias).max() > 0)
    has_bmsg = bool(np.abs(bmsg3).max() > 0)

    # ---- node rows ----
    xb = np.zeros((NROWS, DIM), BF_NP)
    xb[:N_NODES] = x.astype(BF_NP)
    xrow = np.zeros((XR, ROWC), BF_NP)
    xrow[1:AROWS, 0:DIM] = xb[0:ABANK]
    xrow[AROWS + 1:XR, 0:DIM] = xb[ABANK:]
    xt = np.ascontiguousarray(xb.T).astype(BF_NP)        # [128, NROWS]

    # ---- edges: sort by dst, split per window by src bank ----
    order = np.argsort(dst, kind="stable")
    src_s = src[order].astype(np.int64)
    dst_s = dst[order].astype(np.int64)
    ew_s = (edge_emb[eids[order], 0] - ddi[order]).astype(np.float32)
    bounds = np.searchsorted(dst_s, np.arange(0, NROWS + NPW, NPW))

    WN = NCORES * WPC
    winA, winB = [], []
    for wi in range(WN):
        e0, e1 = int(bounds[wi]), int(bounds[wi + 1])
        s_, d_, w_ = src_s[e0:e1], dst_s[e0:e1], ew_s[e0:e1]
        am = s_ < ABANK
        winA.append((s_[am], d_[am], w_[am]))
        winB.append((s_[~am], d_[~am], w_[~am]))

    def kcnt(n):
        return (n + 127) // 128

    # snake-deal windows to cores by descending slot count so each core's
    # rank-j window has a near-identical K, minimizing rank-max padding
    kw = [kcnt(len(winA[g][0])) + kcnt(len(winB[g][0])) for g in range(WN)]
    order_w = np.argsort(-np.asarray(kw), kind="stable")
    asgn = [[] for _ in range(NCORES)]
    for r in range(WPC):
        cores = range(NCORES) if r % 2 == 0 else range(NCORES - 1, -1, -1)
        for i, c in enumerate(cores):
            asgn[c].append(int(order_w[r * NCORES + i]))
    for c in range(NCORES):
        asgn[c].sort(key=lambda g: -kw[g])
    KAs, KBs = [], []
    for j in range(WPC):
        KAs.append(max(kcnt(len(winA[asgn[c][j]][0]))
                       for c in range(NCORES)))
        KBs.append(max(kcnt(len(winB[asgn[c][j]][0]))
                       for c in range(NCORES)))
    kmax = max(a + b for a, b in zip(KAs, KBs))
    sched = (tuple(KAs), tuple(KBs))
    Ks = [a + b for a, b in zip(KAs, KBs)]
    off8 = np.concatenate([[0], np.cumsum([8 * k for k in Ks])]).astype(int)
    off2 = np.concatenate([[0], np.cumsum([2 * k for k in Ks])]).astype(int)
    KTOT = int(sum(Ks))
    assert kmax <= KMAX_TILE

    iota = np.tile(np.arange(128, dtype=np.float32), (128, 1)).astype(BF_NP)
    off128 = np.concatenate([[0], np.cumsum([128 * k for k in Ks])]).astype(int)
    biasr42 = np.tile(sbias, (128, 7)).astype(BF_NP)

    shared = dict(
        xrow=xrow, xt=xt, asd6=asd.astype(BF_NP), biasr42=biasr42,
        wfold=wfold.astype(BF_NP), iota=iota, obias=obias,
        bmsg3=bmsg3)

    in_maps = []
    perm_list = []
    for c in range(NCORES):
        idx_all = np.zeros((128, off8[-1]), np.int16)
        meta_all = np.zeros((128, off2[-1]), np.float32)
        oth_all = np.zeros((128, off128[-1]), E4_NP)
        wnid_a = np.zeros((NPC, 1), np.int32)
        for j in range(WPC):
            gw = asgn[c][j]
            KA, KB, K = KAs[j], KBs[j], Ks[j]
            base = gw * NPW
            dstc = np.full((128, K), 128.0, np.float32)
            ewn = np.zeros((128, K), np.float32)
            idxs = np.zeros(8 * K * 16, np.int64).reshape(16, 8 * K)

            for (s_, d_, w_), k0, rowoff in (
                    (winA[gw], 0, 1),
                    (winB[gw], KA, 1 - ABANK)):
                n = len(s_)
                if n == 0:
                    continue
                i = np.arange(n)
                p = i % 128
                k = k0 + i // 128
                dstc[p, k] = (d_ - base).astype(np.float32)
                ewn[p, k] = w_
                rows = (s_ + rowoff).astype(np.int64)   # bank-local row id
                # gather-local position i -> idx slot [i%16, 8*k0*2 + i//16]
                idxs[i % 16, 8 * k0 + i // 16] = rows
            # Q7 gather reads the idx stream from one 16-partition group
            # (which one depends on the queue) — replicate to all 8 groups.
            idx_all[:, off8[j]:off8[j + 1]] = np.tile(
                idxs.astype(np.int16), (8, 1))
            meta_all[:, off2[j]:off2[j] + K] = dstc
            meta_all[:, off2[j] + K:off2[j] + 2 * K] = ewn
            oth_all[:, off128[j]:off128[j] + K * 128] = (
                np.arange(128)[:, None] == dstc.T.reshape(1, -1)
            ).astype(E4_NP)
            wnid_a[j * 128:(j + 1) * 128, 0] = _row_of(
                np.arange(base, base + NPW))
        m = dict(shared)
        m.update(idx_all=idx_all, meta_all=meta_all, oth_all=oth_all,
                 wnid=wnid_a)
        in_maps.append(m)
        perm_list.append(asgn[c])

    key = (sched, has_obias, has_bmsg)
    return key, in_maps, perm_list


def kernel(**inputs):
    key, in_maps, perm_list = _prep(**inputs)
    sched, has_obias, has_bmsg = key
    kmax = max(a + b for a, b in zip(sched[0], sched[1]))
    if key not in _cache:
        _cache[key] = _build(sched, has_obias, has_bmsg, kmax)
    nc = _cache[key]
    res = bass_utils.run_bass_kernel_spmd(nc, in_maps,
                                          core_ids=list(range(NCORES)))
    out = np.zeros((NROWS, DIM), np.float32)
    for c in range(NCORES):
        oc = res.results[c]["outc"]
        for j in range(WPC):
            gw = int(perm_list[c][j])
            out[gw * NPW:(gw + 1) * NPW] = oc[j * 128:(j + 1) * 128]
    return np.ascontiguousarray(out[:N_NODES]).astype(np.float32)


# revision 28
# speedup vs baseline: 4.8947x; 1.0216x over previous
"""GAT (3-head, edge-weighted) message-passing kernel for 8 Trainium2 NeuronCores.

v2 design (bf16 + dma_gather + folded weights):
  Host folds W_lin into everything: scores s = x @ (W_lin W_h a), messages
  aggregate RAW x and project by W' = W_lin W_h / 3 at the end. Node rows
  live in XROW [50178, 256] bf16 = [x(128) | s_src(3) | s_dst(3) | pad],
  512B rows (dma_gather elem multiple of 256B). Rows are split into two
  banks of <=25089 rows with a poison row 0 each so gather indices fit in
  int16; pad slots index the all-zero poison row.

  Phase 1 (per core, replicated): 8 slab loads of xT bf16, 392 tiny matmuls
  x_tile @ asd6 -> 6 score cols, written into XROW score columns.

  Phase 2: edges sorted by dst; each core owns 49 windows of 128 dst nodes.
  Windows are permuted per-core (sorted by slot count) against a shared
  rank-max schedule so the SPMD instruction stream is identical. Per window:
  one dma_gather per bank pulls all K*128 edge rows; onehot machinery is
  built with 4x-mode bf16 tensor_scalar ops; per 128-edge slot: 3 matmuls
  (agg [128,384], den [128,3], s_dst-broadcast [128,3]) accumulate in PSUM.
  Flush: reciprocal of den, per-head projection by wfold, ACT-scaled sum.
"""

import numpy as np
import ml_dtypes

import concourse.bass as bass
import concourse.bacc as bacc
import concourse.mybir as mybir
from concourse.tile import TileContext
from concourse import bass_utils

F32 = mybir.dt.float32
FP8 = mybir.dt.float8e4
BF16 = mybir.dt.bfloat16
I32 = mybir.dt.int32
I16 = mybir.dt.int16
BF_NP = ml_dtypes.bfloat16
E4_NP = ml_dtypes.float8_e4m3

N_NODES = 50000
N_EDGES = 600000
DIM = 128
NH = 3
NEG = 0.2

NCORES = 8
NPW = 128                    # dst nodes per window
WPC = 49                     # windows per core
NPC = NPW * WPC              # 6272
NTILES = 392                 # node tiles of 128 (50176 nodes incl pad)
NROWS = NTILES * 128         # 50176
ABANK = NROWS // 2           # nodes 0..25087 -> bank A
AROWS = ABANK + 1            # poison row + 25088 node rows
XR = 2 * AROWS               # 50178 total XROW rows
ROWC = 256                   # bf16 cols per row (512B)
SLABS = 8
TPS = NTILES // SLABS        # 49 tiles per slab
KMAX_TILE = 32               # dstcT tile partition allocation

_cache = {}


def _row_of(n):
    """XROW row of node n (vectorized)."""
    n = np.asarray(n)
    return np.where(n < ABANK, 1 + n, AROWS + 1 + (n - ABANK)).astype(np.int64)


def _build(sched, has_obias, has_bmsg, kmax):
    KAs, KBs = sched
    Ks = [a + b for a, b in zip(KAs, KBs)]
    off8 = np.concatenate([[0], np.cumsum([8 * k for k in Ks])]).astype(int)
    off2 = np.concatenate([[0], np.cumsum([2 * k for k in Ks])]).astype(int)
    off128 = np.concatenate([[0], np.cumsum([128 * k for k in Ks])]).astype(int)
    off12 = np.concatenate([[0], np.cumsum([12 * k for k in Ks])]).astype(int)

    nc = bacc.Bacc("TRN2", target_bir_lowering=False, debug=False,
                   num_devices=NCORES)

    xrow = nc.dram_tensor("xrow", [XR, ROWC], BF16, kind="ExternalInput")
    xt = nc.dram_tensor("xt", [128, NROWS], BF16, kind="ExternalInput")
    asd6 = nc.dram_tensor("asd6", [128, 6], BF16, kind="ExternalInput")
    biasr42 = nc.dram_tensor("biasr42", [128, 42], BF16, kind="ExternalInput")
    wfold = nc.dram_tensor("wfold", [128, NH * DIM], BF16,
                           kind="ExternalInput")
    iota = nc.dram_tensor("iota", [128, 128], BF16, kind="ExternalInput")
    obias = nc.dram_tensor("obias", [128, DIM], F32, kind="ExternalInput")
    bmsg3 = nc.dram_tensor("bmsg3", [NH, DIM], F32, kind="ExternalInput")
    # combined per-window metadata: cols [0:8K) idx int16, [8K:12K) meta
    # (dstc|ewn as f32, bitcast to 2x int16 cols)
    cmb_all = nc.dram_tensor("cmb_all", [128, off8[-1] + 2 * off2[-1]], I16,
                             kind="ExternalInput")
    oth_all = nc.dram_tensor("oth_all", [128, off128[-1]], FP8,
                             kind="ExternalInput")
    wnid = nc.dram_tensor("wnid", [NPC, 1], I32, kind="ExternalInput")
    outc = nc.dram_tensor("outc", [NPC, DIM], F32, kind="ExternalOutput")

    with TileContext(nc) as tc:
        with tc.tile_pool(name="const", bufs=1) as cpool:
            asd_sb = cpool.tile([128, 6], BF16, tag="asd")
            nc.sync.dma_start(out=asd_sb[:], in_=asd6[:])
            biasr_sb = cpool.tile([128, 42], BF16, tag="biasr")
            nc.sync.dma_start(out=biasr_sb[:], in_=biasr42[:])
            wf_sb = cpool.tile([128, NH * DIM], BF16, tag="wf")
            nc.sync.dma_start(out=wf_sb[:], in_=wfold[:])
            iota_sb = cpool.tile([128, 128], BF16, tag="iota")
            nc.sync.dma_start(out=iota_sb[:], in_=iota[:])
            obias_sb = cpool.tile([128, DIM], F32, tag="obias")
            if has_obias:
                nc.sync.dma_start(out=obias_sb[:], in_=obias[:])
            bmsg_sb = cpool.tile([NH, DIM], F32, tag="bmsg")
            if has_bmsg:
                nc.sync.dma_start(out=bmsg_sb[:], in_=bmsg3[:])

            # ---------------- phase 1: scores ----------------
            with (
                tc.tile_pool(name="p1", bufs=2) as p1,
                tc.tile_pool(name="p1s", bufs=3) as p1s,
                tc.tile_pool(name="p1ps", bufs=2, space="PSUM") as p1ps,
            ):
                for s in range(SLABS):
                    slab = p1.tile([128, TPS * 128], BF16, tag="slab")
                    nc.sync.dma_start(
                        out=slab[:], in_=xt[:, s * TPS * 128:(s + 1) * TPS * 128])
                    scr = p1s.tile([128, TPS * 6], BF16, tag="scr")
                    for g in range(7):          # 7 groups of 7 tiles
                        ps = p1ps.tile([128, 42], F32, tag="ps")
                        for t in range(7):
                            ti = g * 7 + t
                            nc.tensor.matmul(
                                out=ps[:, 6 * t:6 * t + 6],
                                lhsT=slab[:, ti * 128:(ti + 1) * 128],
                                rhs=asd_sb[:], start=True, stop=True)
                        nc.vector.tensor_tensor(
                            out=scr[:, g * 42:(g + 1) * 42], in0=ps[:],
                            in1=biasr_sb[:], op=mybir.AluOpType.add)
                    row0 = (1 + s * TPS * 128 if s < SLABS // 2
                            else AROWS + 1 + (s - SLABS // 2) * TPS * 128)
                    dst_ap = xrow[row0:row0 + TPS * 128, 128:134].rearrange(
                        "(t p) c -> p t c", p=128)
                    src_ap = scr[:].rearrange("p (t c) -> p t c", c=6)
                    nc.sync.dma_start(out=dst_ap, in_=src_ap)

            # ---------------- phase 2: windows ----------------
            with (
                tc.tile_pool(name="gt", bufs=4) as gpool,
                tc.tile_pool(name="win", bufs=6) as wpool,
                tc.tile_pool(name="ot", bufs=6) as otpool,
                tc.tile_pool(name="sm", bufs=6) as smpool,
                tc.tile_pool(name="mst", bufs=8) as mstpool,
                tc.tile_pool(name="fl", bufs=4) as flpool,
                tc.tile_pool(name="stg", bufs=2) as stgpool,
                tc.tile_pool(name="aggps", bufs=2, space="PSUM") as aggps_p,
                tc.tile_pool(name="denps", bufs=2, space="PSUM") as denps_p,
                tc.tile_pool(name="sdps", bufs=2, space="PSUM") as sdps_p,
                tc.tile_pool(name="ops", bufs=2, space="PSUM") as ops_p,
            ):
                stg = None
                for j in range(WPC):
                    KA, KB = KAs[j], KBs[j]
                    K = KA + KB
                    cmb = wpool.tile([128, 12 * kmax], I16, tag="cmb")
                    nc.sync.dma_start(
                        out=cmb[:, 0:12 * K],
                        in_=cmb_all[:, off12[j]:off12[j] + 12 * K])
                    idxt = cmb[:, 0:8 * K]
                    meta = cmb[:, 8 * K:12 * K].bitcast(F32)
                    oth = wpool.tile([128, kmax * 128], FP8, tag="oth")
                    nc.sync.dma_start(
                        out=oth[:, 0:K * 128],
                        in_=oth_all[:, off128[j]:off128[j] + K * 128])
                    wn = wpool.tile([128, 1], I32, tag="wn")
                    nc.sync.dma_start(out=wn[:], in_=wnid[j * 128:(j + 1) * 128, :])
                    sdw = wpool.tile([128, ROWC], BF16, tag="sdw")
                    nc.gpsimd.indirect_dma_start(
                        out=sdw[:], out_offset=None, in_=xrow[:],
                        in_offset=bass.IndirectOffsetOnAxis(ap=wn[:, 0:1], axis=0))

                    gt = gpool.tile([128, kmax * ROWC], BF16, tag="gt")
                    gv = gt[:, 0:K * ROWC].rearrange("p (k c) -> p k c", c=ROWC)
                    if KA > 0:
                        nc.gpsimd.dma_gather(
                            gv[:, 0:KA, :], xrow[0:AROWS, :],
                            idxt[:, 0:8 * KA], KA * 128, KA * 128, ROWC)
                    if KB > 0:
                        nc.gpsimd.dma_gather(
                            gv[:, KA:K, :], xrow[AROWS:XR, :],
                            idxt[:, 8 * KA:8 * K], KB * 128, KB * 128, ROWC)

                    # s_dst per edge: oT[d,e] = (dstcT[k,e]==d); sd = oT^T @ sdw
                    sdps = sdps_p.tile([128, 3 * kmax], F32, tag="sdps")
                    for k in range(K):
                        nc.tensor.matmul(
                            out=sdps[:, 3 * k:3 * k + 3],
                            lhsT=oth[:, k * 128:(k + 1) * 128],
                            rhs=sdw[:, 131:134], start=True, stop=True)

                    # scores: e = lrelu(s_src + s_dst); p = exp(e); q = p*ew
                    e1 = smpool.tile([128, 3 * kmax], BF16, tag="e1")
                    nc.vector.tensor_tensor(
                        out=e1[:, 0:3 * K].rearrange("p (k c) -> p k c", c=3),
                        in0=gv[:, :, 128:131],
                        in1=sdps[:, 0:3 * K].rearrange("p (k c) -> p k c", c=3),
                        op=mybir.AluOpType.add)
                    sl = smpool.tile([128, 3 * kmax], BF16, tag="sl")
                    nc.vector.tensor_scalar(
                        out=sl[:, 0:3 * K], in0=e1[:, 0:3 * K], scalar1=NEG,
                        scalar2=None, op0=mybir.AluOpType.mult)
                    e2 = smpool.tile([128, 3 * kmax], BF16, tag="e2")
                    nc.vector.tensor_tensor(
                        out=e2[:, 0:3 * K], in0=e1[:, 0:3 * K],
                        in1=sl[:, 0:3 * K], op=mybir.AluOpType.max)
                    p_all = smpool.tile([128, 3 * kmax], BF16, tag="p")
                    nc.scalar.activation(
                        out=p_all[:, 0:3 * K], in_=e2[:, 0:3 * K],
                        func=mybir.ActivationFunctionType.Exp)
                    q_all = smpool.tile([128, 3 * kmax], F32, tag="q")
                    nc.vector.tensor_tensor(
                        out=q_all[:, 0:3 * K].rearrange("p (k c) -> p k c", c=3),
                        in0=p_all[:, 0:3 * K].rearrange("p (k c) -> p k c", c=3),
                        in1=meta[:, K:2 * K].unsqueeze(2).broadcast_to(
                            (128, K, 3)),
                        op=mybir.AluOpType.mult)

                    aggps = aggps_p.tile([128, NH * 128], F32, tag="agg")
                    denps = denps_p.tile([128, 8], F32, tag="den")
                    for k in range(K):
                        mst = mstpool.tile([128, 512], BF16, tag="mst")
                        dcol = meta[:, k:k + 1]
                        nc.vector.tensor_scalar(
                            out=mst[:, 384:512], in0=iota_sb[:],
                            scalar1=dcol, scalar2=None,
                            op0=mybir.AluOpType.is_equal)
                        for h in range(2):
                            nc.vector.tensor_scalar(
                                out=mst[:, h * 128:(h + 1) * 128],
                                in0=iota_sb[:], scalar1=dcol,
                                scalar2=q_all[:, 3 * k + h:3 * k + h + 1],
                                op0=mybir.AluOpType.is_equal,
                                op1=mybir.AluOpType.mult)
                        if k % 8 == 7:
                            nc.vector.tensor_scalar(
                                out=mst[:, 256:384], in0=iota_sb[:],
                                scalar1=dcol,
                                scalar2=q_all[:, 3 * k + 2:3 * k + 3],
                                op0=mybir.AluOpType.is_equal,
                                op1=mybir.AluOpType.mult)
                        else:
                            nc.scalar.activation(
                                out=mst[:, 256:384], in_=mst[:, 384:512],
                                func=mybir.ActivationFunctionType.Copy,
                                scale=q_all[:, 3 * k + 2:3 * k + 3])
                        nc.tensor.matmul(
                            out=aggps[:], lhsT=gt[:, k * ROWC:k * ROWC + 128],
                            rhs=mst[:, 0:384], start=(k == 0), stop=(k == K - 1))
                        nc.tensor.matmul(
                            out=denps[:, 0:3], lhsT=mst[:, 384:512],
                            rhs=p_all[:, 3 * k:3 * k + 3],
                            start=(k == 0), stop=(k == K - 1))
                        if has_bmsg:
                            nc.tensor.matmul(
                                out=denps[:, 3:6], lhsT=mst[:, 384:512],
                                rhs=q_all[:, 3 * k:3 * k + 3],
                                start=(k == 0), stop=(k == K - 1))

                    # ---- window flush ----
                    dinv = flpool.tile([128, 3], F32, tag="dinv")
                    nc.vector.tensor_scalar(
                        out=dinv[:], in0=denps[:, 0:3], scalar1=1e-16,
                        scalar2=None, op0=mybir.AluOpType.max)
                    nc.vector.reciprocal(out=dinv[:], in_=dinv[:])
                    aggsb = flpool.tile([128, NH * 128], BF16, tag="aggsb")
                    if j % 2 == 0:
                        nc.scalar.activation(
                            out=aggsb[:], in_=aggps[:],
                            func=mybir.ActivationFunctionType.Copy)
                    else:
                        nc.vector.tensor_scalar(
                            out=aggsb[:], in0=aggps[:], scalar1=0.0,
                            scalar2=None, op0=mybir.AluOpType.add)
                    ths = []
                    for h in range(NH):
                        o2 = ops_p.tile([128, DIM], F32, tag="o2")
                        nc.tensor.matmul(
                            out=o2[:], lhsT=aggsb[:, h * 128:(h + 1) * 128],
                            rhs=wf_sb[:, h * DIM:(h + 1) * DIM],
                            start=True, stop=True)
                        th = flpool.tile([128, DIM], F32, tag=f"th{h}")
                        nc.scalar.activation(
                            out=th[:], in_=o2[:],
                            func=mybir.ActivationFunctionType.Copy,
                            scale=dinv[:, h:h + 1])
                        ths.append(th)
                    if j % 7 == 0:
                        stg = stgpool.tile([128, 7 * DIM], F32, tag="stg")
                    sg = stg[:, (j % 7) * DIM:(j % 7 + 1) * DIM]
                    a01 = flpool.tile([128, DIM], F32, tag="a01")
                    nc.vector.tensor_tensor(out=a01[:], in0=ths[0][:],
                                            in1=ths[1][:],
                                            op=mybir.AluOpType.add)
                    if has_bmsg:
                        a01b = flpool.tile([128, DIM], F32, tag="a01b")
                        nc.vector.tensor_tensor(out=a01b[:], in0=a01[:],
                                                in1=ths[2][:],
                                                op=mybir.AluOpType.add)
                        tn = flpool.tile([128, 3], F32, tag="tn")
                        nc.vector.tensor_tensor(out=tn[:], in0=denps[:, 3:6],
                                                in1=dinv[:],
                                                op=mybir.AluOpType.mult)
                        bc = flpool.tile([128, DIM], F32, tag="bc")
                        for h in range(NH):
                            dst_t = bc if h == 0 else flpool.tile(
                                [128, DIM], F32, tag="bch")
                            nc.vector.tensor_scalar(
                                out=dst_t[:],
                                in0=bmsg_sb[h:h + 1, :].partition_broadcast(128),
                                scalar1=tn[:, h:h + 1], scalar2=None,
                                op0=mybir.AluOpType.mult)
                            if h > 0:
                                nc.vector.tensor_tensor(
                                    out=bc[:], in0=bc[:], in1=dst_t[:],
                                    op=mybir.AluOpType.add)
                        if has_obias:
                            nc.vector.tensor_tensor(
                                out=bc[:], in0=bc[:], in1=obias_sb[:],
                                op=mybir.AluOpType.add)
                        nc.vector.tensor_tensor(out=sg, in0=a01b[:],
                                                in1=bc[:],
                                                op=mybir.AluOpType.add)
                    else:
                        if has_obias:
                            a2 = flpool.tile([128, DIM], F32, tag="a2")
                            nc.vector.tensor_tensor(out=a2[:], in0=ths[2][:],
                                                    in1=obias_sb[:],
                                                    op=mybir.AluOpType.add)
                            nc.vector.tensor_tensor(out=sg, in0=a01[:],
                                                    in1=a2[:],
                                                    op=mybir.AluOpType.add)
                        else:
                            nc.vector.tensor_tensor(out=sg, in0=a01[:],
                                                    in1=ths[2][:],
                                                    op=mybir.AluOpType.add)
                    if j % 7 == 6:
                        j0 = j - 6
                        dst_ap = outc[j0 * 128:(j0 + 7) * 128, :].rearrange(
                            "(g p) c -> p g c", p=128)
                        nc.sync.dma_start(
                            out=dst_ap,
                            in_=stg[:].rearrange("p (g c) -> p g c", c=DIM))

    nc.compile()
    return nc


def _prep(x, edge_index, edge_ids, ddi_weight, W_lin, b_lin, edge_emb,
          W_heads, att_src, att_dst, bias_heads):
    x = np.asarray(x, np.float32)
    src = np.asarray(edge_index[0], np.int64)
    dst = np.asarray(edge_index[1], np.int64)
    eids = np.asarray(edge_ids, np.int64)
    ddi = np.asarray(ddi_weight, np.float32)
    W_lin = np.asarray(W_lin, np.float32)
    b_lin = np.asarray(b_lin, np.float32)
    edge_emb = np.asarray(edge_emb, np.float32)
    W_heads = np.asarray(W_heads, np.float32)
    att_src = np.asarray(att_src, np.float32)
    att_dst = np.asarray(att_dst, np.float32)
    bias_heads = np.asarray(bias_heads, np.float32)

    # ---- folded weights ----
    asd = np.zeros((DIM, 6), np.float32)
    sbias = np.zeros(6, np.float32)
    for h in range(NH):
        wsrc = W_lin @ W_heads[h] @ att_src[h]
        wdst = W_lin @ W_heads[h] @ att_dst[h]
        asd[:, h] = wsrc
        asd[:, 3 + h] = wdst
        sbias[h] = b_lin @ W_heads[h] @ att_src[h]
        sbias[3 + h] = b_lin @ W_heads[h] @ att_dst[h]
    wfold = np.zeros((DIM, NH * DIM), np.float32)
    bmsg3 = np.zeros((NH, DIM), np.float32)
    for h in range(NH):
        wfold[:, h * DIM:(h + 1) * DIM] = (W_lin @ W_heads[h]) / NH
        bmsg3[h] = (b_lin @ W_heads[h]) / NH
    obias = np.tile(bias_heads.sum(0) / NH, (128, 1)).astype(np.float32)
    has_obias = bool(np.abs(obias).max() > 0)
    has_bmsg = bool(np.abs(bmsg3).max() > 0)

    # ---- node rows ----
    xb = np.zeros((NROWS, DIM), BF_NP)
    xb[:N_NODES] = x.astype(BF_NP)
    xrow = np.zeros((XR, ROWC), BF_NP)
    xrow[1:AROWS, 0:DIM] = xb[0:ABANK]
    xrow[AROWS + 1:XR, 0:DIM] = xb[ABANK:]
    xt = np.ascontiguousarray(xb.T).astype(BF_NP)        # [128, NROWS]

    # ---- edges: sort by dst, split per window by src bank ----
    order = np.argsort(dst, kind="stable")
    src_s = src[order].astype(np.int64)
    dst_s = dst[order].astype(np.int64)
    ew_s = (edge_emb[eids[order], 0] - ddi[order]).astype(np.float32)
    bounds = np.searchsorted(dst_s, np.arange(0, NROWS + NPW, NPW))

    WN = NCORES * WPC
    winA, winB = [], []
    for wi in range(WN):
        e0, e1 = int(bounds[wi]), int(bounds[wi + 1])
        s_, d_, w_ = src_s[e0:e1], dst_s[e0:e1], ew_s[e0:e1]
        am = s_ < ABANK
        winA.append((s_[am], d_[am], w_[am]))
        winB.append((s_[~am], d_[~am], w_[~am]))

    def kcnt(n):
        return (n + 127) // 128

    # snake-deal windows to cores by descending slot count so each core's
    # rank-j window has a near-identical K, minimizing rank-max padding
    kab = [(kcnt(len(winA[g][0])), kcnt(len(winB[g][0]))) for g in range(WN)]
    order_w = sorted(range(WN),
                     key=lambda g: (-(kab[g][0] + kab[g][1]), -kab[g][0]))
    # consecutive groups of 8 share a rank -> near-identical (KA, KB) pairs
    asgn = [[int(order_w[r * NCORES + c]) for r in range(WPC)]
            for c in range(NCORES)]
    KAs, KBs = [], []
    for j in range(WPC):
        KAs.append(max(kcnt(len(winA[asgn[c][j]][0]))
                       for c in range(NCORES)))
        KBs.append(max(kcnt(len(winB[asgn[c][j]][0]))
                       for c in range(NCORES)))
    kmax = max(a + b for a, b in zip(KAs, KBs))
    sched = (tuple(KAs), tuple(KBs))
    Ks = [a + b for a, b in zip(KAs, KBs)]
    off8 = np.concatenate([[0], np.cumsum([8 * k for k in Ks])]).astype(int)
    off2 = np.concatenate([[0], np.cumsum([2 * k for k in Ks])]).astype(int)
    KTOT = int(sum(Ks))
    assert kmax <= KMAX_TILE

    iota = np.tile(np.arange(128, dtype=np.float32), (128, 1)).astype(BF_NP)
    off128 = np.concatenate([[0], np.cumsum([128 * k for k in Ks])]).astype(int)
    biasr42 = np.tile(sbias, (128, 7)).astype(BF_NP)

    shared = dict(
        xrow=xrow, xt=xt, asd6=asd.astype(BF_NP), biasr42=biasr42,
        wfold=wfold.astype(BF_NP), iota=iota, obias=obias,
        bmsg3=bmsg3)

    in_maps = []
    perm_list = []
    for c in range(NCORES):
        off12 = np.concatenate([[0], np.cumsum([12 * k for k in Ks])]).astype(int)
        cmb_all = np.zeros((128, off12[-1]), np.int16)
        oth_all = np.zeros((128, off128[-1]), E4_NP)
        wnid_a = np.zeros((NPC, 1), np.int32)
        for j in range(WPC):
            gw = asgn[c][j]
            KA, KB, K = KAs[j], KBs[j], Ks[j]
            base = gw * NPW
            dstc = np.full((128, K), 128.0, np.float32)
            ewn = np.zeros((128, K), np.float32)
            idxs = np.zeros(8 * K * 16, np.int64).reshape(16, 8 * K)

            for (s_, d_, w_), k0, rowoff in (
                    (winA[gw], 0, 1),
                    (winB[gw], KA, 1 - ABANK)):
                n = len(s_)
                if n == 0:
                    continue
                i = np.arange(n)
                p = i % 128
                k = k0 + i // 128
                dstc[p, k] = (d_ - base).astype(np.float32)
                ewn[p, k] = w_
                rows = (s_ + rowoff).astype(np.int64)   # bank-local row id
                # gather-local position i -> idx slot [i%16, 8*k0*2 + i//16]
                idxs[i % 16, 8 * k0 + i // 16] = rows
            # Q7 gather reads the idx stream from one 16-partition group
            # (which one depends on the queue) — replicate to all 8 groups.
            cmb_all[:, off12[j]:off12[j] + 8 * K] = np.tile(
                idxs.astype(np.int16), (8, 1))
            meta32 = np.concatenate([dstc, ewn], axis=1).astype(np.float32)
            cmb_all[:, off12[j] + 8 * K:off12[j] + 12 * K] = \
                meta32.view(np.int16)
            oth_all[:, off128[j]:off128[j] + K * 128] = (
                np.arange(128)[:, None] == dstc.T.reshape(1, -1)
            ).astype(E4_NP)
            wnid_a[j * 128:(j + 1) * 128, 0] = _row_of(
                np.arange(base, base + NPW))
        m = dict(shared)
        m.update(cmb_all=cmb_all, oth_all=oth_all,
                 wnid=wnid_a)
        in_maps.append(m)
        perm_list.append(asgn[c])

    key = (sched, has_obias, has_bmsg)
    return key, in_maps, perm_list


def kernel(**inputs):
    key, in_maps, perm_list = _prep(**inputs)
    sched, has_obias, has_bmsg = key
    kmax = max(a + b for a, b in zip(sched[0], sched[1]))
    if key not in _cache:
        _cache[key] = _build(sched, has_obias, has_bmsg, kmax)
    nc = _cache[key]
    res = bass_utils.run_bass_kernel_spmd(nc, in_maps,
                                          core_ids=list(range(NCORES)))
    out = np.zeros((NROWS, DIM), np.float32)
    for c in range(NCORES):
        oc = res.results[c]["outc"]
        for j in range(WPC):
            gw = int(perm_list[c][j])
            out[gw * NPW:(gw + 1) * NPW] = oc[j * 128:(j + 1) * 128]
    return np.ascontiguousarray(out[:N_NODES]).astype(np.float32)
